# revision 13
# baseline (speedup 1.0000x reference)
"""Trainium2 Bass kernel for nn_MemoryGraph (gnn_message_passing).

Sharding: neurons split across 8 cores (1024/core), all 4 batches local.
msgs stored batch-interleaved [n, b, d] in fp32 so ONE 1KB gather
descriptor per edge (n,k) serves all 4 batches; one full-world AllGather
per step (skipped on the last step; step-0 reads a host-built full
buffer, so there is no preamble collective).

Per step (per core, R = 4*1024 rows, cols r = b*1024 + n, feature-major):
  - gather neighbor msg rows from DRAM mfull [8192, 4*64]
  - received = per-target K-weighted sums as tiny PE matmuls
    (stationary = gathered [32k x 64d] block, moving = block-diag w column)
  - MLP matmuls in float32r (1 cycle/row at >=256 free; dst partition 0)
  - mod MLP2 emitted feature-major: ident delta first (per gather phase,
    it gates the state MLP), then 1-decay via replicated-column
    stationary, then w^T wrapped for the next step's tiny-mm movings
  - h kept fp32 in Hf (recurrence noise is amplified ~2x/step, so the
    fed-back state must stay >= f32r precision; bf16/fp16 fail), mirrored
    into f32r C[0:64] for matmuls; output DMA'd as [T, 64, R]
Host side: layout prep in numpy; output reassembly at the end.
"""

import numpy as np
import ml_dtypes

import concourse.bass as bass
import concourse.bacc as bacc
from concourse import mybir, tile, library_config
from concourse.bass_utils import run_bass_kernel_spmd

# problem constants (hardcoded per harness contract)
N, K, D, D_ID = 8192, 32, 64, 32
H = 256
BS, T = 4, 8
NCORES = 8
NB = BS                   # batches per core (all four)
NT = N // NCORES          # 1024 neurons (targets) per core
R = NB * NT               # 4096 rows per core (cols r = b*NT + n_loc)
NG = 32                   # gather instructions per step (1024 idx each)
EL = NB * D               # gathered elem: 256 f32 = 1024B (all batches)

F32 = mybir.dt.float32
F32R = mybir.dt.float32r
BF16 = mybir.dt.bfloat16
F16 = mybir.dt.float16
I16 = mybir.dt.int16
AF = mybir.ActivationFunctionType
ALU = mybir.AluOpType

_PROGRAM_CACHE = {}


def _build_program():
    nc = bacc.Bacc(
        "TRN2", target_bir_lowering=False, debug=False,
        num_devices=NCORES,
    )

    din = {}
    def dram_in(name, shape, dtype=F32):
        din[name] = nc.dram_tensor(name, shape, dtype, kind="ExternalInput")
        return din[name]

    h0T = dram_in("h0T", [D, R])
    hebbT = dram_in("hebbT", [D_ID, R], F32R)
    identT_in = dram_in("identT", [D_ID, NT])
    injT = dram_in("injT", [T, D, R], F32R)
    mfull0 = dram_in("mfull0", [N, EL], F16)
    w0blk = dram_in("w0blk", [128, 4, NB, NT // 4], F16)
    idx_in = dram_in("idx", [128, 2048], I16)
    dw1C = dram_in("dw1C", [128, H], F32R)
    dw1B = dram_in("dw1B", [128, H], F32R)
    db1 = dram_in("db1", [128, 2])
    dw2wT = dram_in("dw2wT", [128, 2, K], F32R)
    db2wT = dram_in("db2wT", [K, 1])
    dw2om = dram_in("dw2om", [128, 2, D], F32R)
    db2om = dram_in("db2om", [D, 1])
    dw2de = dram_in("dw2de", [128, 2, D_ID], F32R)
    db2de = dram_in("db2de", [D_ID, 1])  # pre-scaled x2 for the 2-core AG
    sw1B = dram_in("sw1B", [128, H], F32R)
    sw1C = dram_in("sw1C", [96, H], F32R)
    sb1 = dram_in("sb1", [128, 2])
    sw2 = dram_in("sw2", [128, 2, D], F32R)
    sb2 = dram_in("sb2", [D, 1])
    mw1 = dram_in("mw1", [96, H], F32R)
    mb1 = dram_in("mb1", [128, 2])
    mw2 = dram_in("mw2", [128, 2, D], F32R)
    mb2 = dram_in("mb2", [1, D], F32R)
    ones1 = dram_in("ones1", [1, 128], F32R)

    out_d = nc.dram_tensor("out", [T, D, R], F32, kind="ExternalOutput")

    rg_msgs = [list(range(NCORES))]

    with tile.TileContext(nc) as tc:
        with (
            tc.tile_pool(name="persist", bufs=1) as pp,
            tc.tile_pool(name="dram", bufs=1, space="DRAM") as dp,
            tc.tile_pool(name="gpool", bufs=2) as gp,
            tc.tile_pool(name="hid", bufs=1) as hp,
            tc.tile_pool(name="psR", bufs=4, space="PSUM") as psRp,
            tc.tile_pool(name="ps1", bufs=4, space="PSUM") as ps1p,
        ):
            mshard = dp.tile([NT, EL], F16, name="mshard", tag="mshard")
            mfull = dp.tile([N, EL], F16, name="mfull", tag="mfull")

            # persistent SBUF
            C = pp.tile([128, R], F32R)        # [h(64); ide(32); hebb(32)]
            B = pp.tile([128, R], F32R)        # [received(64); inject(64)]
            wTblk = pp.tile([128, 4, NB, NT // 4], F16)   # block-diag w^T
            identM = pp.tile([D_ID, NT], F32)
            omR = pp.tile([D, R], F32)        # (1-decay) replicated over d
            Ttanh = pp.tile([D, R], F32)
            Hf = pp.tile([D, R], F32)         # fp32 master copy of h
            dI = pp.tile([D_ID, 2, NT], F32)  # per-batch ident deltas
            msgs = pp.tile([128, NT // 128, EL], F16)
            idxT = pp.tile([128, 2048], I16)
            onesK = pp.tile([1, 128], F32R)
            onesK_src = onesK  # DMA'd from ones1 input
            # weights
            t_dw1C = pp.tile([128, H], F32R)
            t_dw1B = pp.tile([128, H], F32R)
            t_db1 = pp.tile([128, 2], F32)
            t_dw2wT = pp.tile([128, 2, K], F32R)
            t_db2wT = pp.tile([K, 1], F32)
            t_dw2om = pp.tile([128, 2, D], F32R)
            t_db2om = pp.tile([D, 1], F32)
            t_dw2de = pp.tile([128, 2, D_ID], F32R)
            t_db2de = pp.tile([D_ID, 1], F32)
            t_sw1B = pp.tile([128, H], F32R)
            t_sw1C = pp.tile([96, H], F32R)
            t_sb1 = pp.tile([128, 2], F32)
            t_sw2 = pp.tile([128, 2, D], F32R)
            t_sb2 = pp.tile([D, 1], F32)
            t_mw1 = pp.tile([96, H], F32R)
            t_mb1 = pp.tile([128, 2], F32)
            t_mw2 = pp.tile([128, 2, D], F32R)
            t_mb2 = pp.tile([1, D], F32R)

            # ---------------- preamble ----------------
            nc.gpsimd.load_library(library_config.mlp)

            for tname, ttile in [
                ("dw1C", t_dw1C), ("dw1B", t_dw1B), ("db1", t_db1),
                ("dw2wT", t_dw2wT), ("db2wT", t_db2wT),
                ("dw2om", t_dw2om), ("db2om", t_db2om),
                ("dw2de", t_dw2de), ("db2de", t_db2de),
                ("sw1B", t_sw1B), ("sw1C", t_sw1C), ("sb1", t_sb1),
                ("sw2", t_sw2), ("sb2", t_sb2),
                ("mw1", t_mw1), ("mb1", t_mb1),
                ("mw2", t_mw2), ("mb2", t_mb2), ("ones1", onesK_src),
            ]:
                nc.sync.dma_start(out=ttile[:], in_=din[tname][:])

            nc.sync.dma_start(out=Hf[:], in_=h0T[:])
            nc.scalar.copy(out=C[0:D, :], in_=Hf[:])
            nc.sync.dma_start(out=C[96:128, :], in_=hebbT[:])
            nc.sync.dma_start(out=identM[:], in_=identT_in[:])
            nc.sync.dma_start(out=wTblk[:], in_=w0blk[:])
            nc.sync.dma_start(out=idxT[:], in_=idx_in[:])
            ide_b = identM[:].unsqueeze(1).broadcast_to([D_ID, NB, NT])
            nc.scalar.copy(
                out=C[D:96, :].rearrange("p (b n) -> p b n", b=NB),
                in_=ide_b)
            # ---------------- time loop ----------------
            for t in range(T):
                nc.sync.dma_start(out=B[D:2 * D, :], in_=injT[t])

                # ---- received: gather + tiny weighted matmuls ----
                modH = hp.tile([128, 2, R], F32R, tag="hid")
                for ph in range(2):
                    pR = [psRp.tile([64, 512], F32, tag="psR",
                                    name=f"pR{ph}_{_b}")
                          for _b in range(NB)]
                    for gi in range(16):
                        g = 16 * ph + gi
                        G = gp.tile([128, 8, EL], F16, tag="G")
                        nc.gpsimd.dma_gather(
                            out_ap=G[:],
                            in_ap=(mfull0[:] if t == 0 else mfull[:]),
                            idxs_ap=idxT[:, 64 * g:64 * (g + 1)],
                            num_idxs=1024,
                            num_idxs_reg=1024,
                            elem_size=EL,
                        )
                        for cp in range(8):
                            col = 4 * (8 * gi + cp)
                            c_glob = 8 * g + cp
                            for b in range(NB):
                                nc.tensor.matmul(
                                    pR[b][:, col:col + 4],
                                    G[:, cp, D * b:D * (b + 1)],
                                    wTblk[:, :, b, c_glob],
                                    start=True, stop=True,
                                    skip_group_check=True)
                    # drain received -> B rows 0:64 (b-major cols)
                    for b in range(NB):
                        nc.scalar.copy(
                            out=B[0:D, NT * b + 512 * ph:
                                  NT * b + 512 * (ph + 1)],
                            in_=pR[b][:])
                    # mod MLP1 for this phase's drained chunks
                    for b in range(NB):
                        sl = slice(NT * b + 512 * ph,
                                   NT * b + 512 * (ph + 1))
                        for m in range(2):
                            ps = ps1p.tile([128, 512], F32, tag="mm1")
                            nc.tensor.matmul(
                                ps[:], t_dw1C[:, 128 * m:128 * (m + 1)],
                                C[:, sl], start=True, stop=False)
                            nc.tensor.matmul(
                                ps[:], t_dw1B[:, 128 * m:128 * (m + 1)],
                                B[:, sl], start=False, stop=True)
                            nc.scalar.activation(
                                out=modH[:, m, sl], in_=ps[:], func=AF.Silu,
                                bias=t_db1[:, m:m + 1])
                    # ident delta for this phase's columns (q = ph)
                    for b in range(NB):
                        slot = 0 if b == 0 else 1
                        dsl = slice(NT * b + 512 * ph,
                                    NT * b + 512 * (ph + 1))
                        csl = slice(512 * ph, 512 * (ph + 1))
                        ps = ps1p.tile([128, 512], F32, tag="mm1")
                        for m in range(2):
                            nc.tensor.matmul(
                                ps[0:D_ID, :], t_dw2de[:, m, :],
                                modH[:, m, dsl],
                                start=(m == 0), stop=(m == 1),
                                skip_group_check=True)
                        nc.scalar.activation(
                            out=dI[:, slot, csl],
                            in_=ps[0:D_ID, :], func=AF.Identity,
                            bias=(t_db2de[:] if b == 0 else 0.0))
                        if b > 0:
                            nc.vector.tensor_tensor(
                                out=dI[:, 0, csl], in0=dI[:, 0, csl],
                                in1=dI[:, 1, csl], op=ALU.add)

                # ---- mod MLP2 (feature-major outputs) ----
                modHr = modH[:].rearrange("p m (b c g) -> p m b c g",
                                          b=NB, g=4)
                # om = 1 - decay, replicated over 64 partitions via
                # column-replicated stationary
                for q in range(8):
                    sl = slice(512 * q, 512 * (q + 1))
                    ps = ps1p.tile([128, 512], F32, tag="mm1")
                    for m in range(2):
                        nc.tensor.matmul(
                            ps[0:D, :], t_dw2om[:, m, :],
                            modH[:, m, sl],
                            start=(m == 0), stop=(m == 1),
                            skip_group_check=True)
                    nc.scalar.activation(
                        out=omR[:, sl], in_=ps[0:D, :], func=AF.Sigmoid,
                        scale=-1.0, bias=t_db2om[:])
                if t < T - 1:
                    # w^T wrapped: wTblk[32g+k, g, b, c] = sig(w[b, 4c+g, k])
                    for g in range(4):
                        for ch in range(2):
                            ps = ps1p.tile([128, 512], F32, tag="mm1")
                            for bi in range(2):
                                b = 2 * ch + bi
                                for m in range(2):
                                    nc.tensor.matmul(
                                        ps[0:K, 256 * bi:256 * (bi + 1)],
                                        t_dw2wT[:, m, :],
                                        modHr[:, m, b, :, g],
                                        start=(m == 0), stop=(m == 1),
                                        skip_group_check=True)
                            nc.scalar.activation(
                                out=wTblk[32 * g:32 * (g + 1), g,
                                          2 * ch:2 * (ch + 1), :],
                                in_=ps[0:K, :].rearrange("p (b c) -> p b c", b=2),
                                func=AF.Sigmoid, bias=t_db2wT[:])
                # ---- ident update + broadcast to C rows 64:96 ----
                nc.vector.scalar_tensor_tensor(
                    out=identM[:], in0=dI[:, 0, :], scalar=1.0 / BS,
                    in1=identM[:], op0=ALU.mult, op1=ALU.add)
                ide_b2 = identM[:].unsqueeze(1).broadcast_to([D_ID, NB, NT])
                nc.scalar.copy(
                    out=C[D:96, :].rearrange("p (b n) -> p b n", b=NB),
                    in_=ide_b2)

                # ---- state / h / msg, pipelined by neuron halves ----
                # half nh covers neurons 512*nh..512*(nh+1) of every batch
                stateH = hp.tile([128, 2, R], F32R, tag="hid")
                for nh in range(2):
                    for b in range(NB):
                        sl = slice(NT * b + 512 * nh,
                                   NT * b + 512 * (nh + 1))
                        for m in range(2):
                            ps = ps1p.tile([128, 512], F32, tag="mm1")
                            nc.tensor.matmul(
                                ps[:], t_sw1B[:, 128 * m:128 * (m + 1)],
                                B[:, sl], start=True, stop=False)
                            nc.tensor.matmul(
                                ps[:], t_sw1C[:, 128 * m:128 * (m + 1)],
                                C[0:96, sl], start=False, stop=True)
                            nc.scalar.activation(
                                out=stateH[:, m, sl], in_=ps[:],
                                func=AF.Silu, bias=t_sb1[:, m:m + 1])
                    for b in range(NB):
                        sl = slice(NT * b + 512 * nh,
                                   NT * b + 512 * (nh + 1))
                        ps = ps1p.tile([128, 512], F32, tag="mm1")
                        for m in range(2):
                            nc.tensor.matmul(
                                ps[0:D, :], t_sw2[:, m, :],
                                stateH[:, m, sl],
                                start=(m == 0), stop=(m == 1),
                                skip_group_check=True)
                        nc.scalar.activation(
                            out=Ttanh[:, sl], in_=ps[0:D, :], func=AF.Tanh,
                            bias=t_sb2[:])

                if t < T - 1:
                    msgH = hp.tile([128, 2, R], F32R, tag="hid")
                Hv = Hf[:].rearrange("p (b n) -> p b n", b=NB)
                Tv = Ttanh[:].rearrange("p (b n) -> p b n", b=NB)
                Ov = omR[:].rearrange("p (b n) -> p b n", b=NB)
                Cv = C[0:D, :].rearrange("p (b n) -> p b n", b=NB)
                for nh in range(2):
                    s = slice(512 * nh, 512 * (nh + 1))
                    e1, e2 = ((nc.gpsimd, nc.vector) if nh == 0
                              else (nc.vector, nc.gpsimd))
                    # h_new = h + om*(tanh - h) for this half's columns
                    e1.tensor_tensor(out=Tv[:, :, s], in0=Tv[:, :, s],
                                     in1=Hv[:, :, s], op=ALU.subtract)
                    e1.tensor_tensor(out=Tv[:, :, s], in0=Tv[:, :, s],
                                     in1=Ov[:, :, s], op=ALU.mult)
                    e2.tensor_tensor(out=Hv[:, :, s], in0=Hv[:, :, s],
                                     in1=Tv[:, :, s], op=ALU.add)
                    nc.scalar.copy(out=Cv[:, :, s], in_=Hv[:, :, s])
                    if t == T - 1:
                        continue
                    # msg MLP for this half, while the other half's
                    # state/h work occupies the remaining engines
                    for b in range(NB):
                        sl = slice(NT * b + 512 * nh,
                                   NT * b + 512 * (nh + 1))
                        for m in range(2):
                            ps = ps1p.tile([128, 512], F32, tag="mm1")
                            nc.tensor.matmul(
                                ps[:], t_mw1[:, 128 * m:128 * (m + 1)],
                                C[0:96, sl], start=True, stop=True)
                            nc.scalar.activation(
                                out=msgH[:, m, sl], in_=ps[:],
                                func=AF.Silu, bias=t_mb1[:, m:m + 1])
                    for j in range(4 * nh, 4 * (nh + 1)):
                        ps = ps1p.tile([128, 512], F32, tag="mm1")
                        for b in range(NB):
                            rsl = slice(NT * b + 128 * j,
                                        NT * b + 128 * (j + 1))
                            nc.tensor.matmul(
                                ps[:, D * b:D * (b + 1)], msgH[:, 0, rsl],
                                t_mw2[:, 0, :], start=True, stop=False,
                                skip_group_check=True)
                            nc.tensor.matmul(
                                ps[:, D * b:D * (b + 1)], msgH[:, 1, rsl],
                                t_mw2[:, 1, :], start=False, stop=False,
                                skip_group_check=True)
                            nc.tensor.matmul(
                                ps[:, D * b:D * (b + 1)], onesK[:],
                                t_mb2[:], start=False, stop=True,
                                skip_group_check=True)
                        nc.scalar.activation(
                            out=msgs[:, j, :], in_=ps[:, 0:EL], func=AF.Tanh)
                    # this half's msgs rows -> DRAM shard piece
                    nc.sync.dma_start(
                        out=mshard[512 * nh:512 * (nh + 1), :]
                        .rearrange("(j p) d -> p j d", p=128),
                        in_=msgs[:, 4 * nh:4 * (nh + 1), :])

                # output word_states for this step (feature-major)
                nc.sync.dma_start(out=out_d[t], in_=Hf[:])
                if t == T - 1:
                    continue
                nc.gpsimd.collective_compute(
                    "AllGather", ALU.bypass, ins=[mshard.opt()],
                    outs=[mfull.opt()], replica_groups=rg_msgs)

    nc.finalize()
    return nc


def _prep_inputs(inputs):
    """Build the per-core input maps from the full problem inputs."""
    cc = np.asarray(inputs["cc_signals"], dtype=np.float32)
    h0 = np.asarray(inputs["h0"], dtype=np.float32)
    msgs0 = np.asarray(inputs["msgs0"], dtype=np.float32)
    w_conn0 = np.asarray(inputs["w_conn0"], dtype=np.float32)
    hebb = np.asarray(inputs["hebbian"], dtype=np.float32)
    ident = np.asarray(inputs["identity"], dtype=np.float32)
    conn = np.asarray(inputs["conn_indices"]).astype(np.int64)

    def f32(x):
        return np.ascontiguousarray(x, dtype=np.float32)

    def bf16(x):
        return np.ascontiguousarray(
            np.asarray(x, dtype=np.float32).astype(ml_dtypes.bfloat16))

    dw1 = np.asarray(inputs["dw1"], dtype=np.float32)   # [256, 256]
    dw2 = np.asarray(inputs["dw2"], dtype=np.float32)   # [256, 65]
    db2 = np.asarray(inputs["db2"], dtype=np.float32)   # [65]
    sw1 = np.asarray(inputs["sw1"], dtype=np.float32)   # [224, 256]
    sw2 = np.asarray(inputs["sw2"], dtype=np.float32)   # [256, 64]
    mw1 = np.asarray(inputs["mw1"], dtype=np.float32)   # [96, 256]
    mw2 = np.asarray(inputs["mw2"], dtype=np.float32)   # [256, 64]

    # dw1 input order: [hebb(0:32), h(32:96), ide(96:128), rcv, inj]
    # C rows: [h, ide, hebb]; B rows: [rcv, inj]
    shared = {
        "dw1C": f32(np.concatenate([dw1[32:96], dw1[96:128], dw1[0:32]])),
        "dw1B": f32(dw1[128:256]),
        "db1": f32(np.asarray(inputs["db1"]).reshape(2, 128).T),
        "dw2wT": f32(dw2[:, 0:K].reshape(2, 128, K).transpose(1, 0, 2)),
        "db2wT": f32(db2[0:K].reshape(K, 1)),
        "dw2om": f32(np.repeat(dw2[:, K:K + 1], D, axis=1)
                      .reshape(2, 128, D).transpose(1, 0, 2)),
        "db2om": f32(np.full((D, 1), -db2[K])),
        "dw2de": f32(dw2[:, K + 1:].reshape(2, 128, D_ID).transpose(1, 0, 2)),
        "db2de": f32(4.0 * db2[K + 1:].reshape(D_ID, 1)),
        "sw1B": f32(sw1[0:128]),
        "sw1C": f32(sw1[128:224]),
        "sb1": f32(np.asarray(inputs["sb1"]).reshape(2, 128).T),
        "sw2": f32(sw2.reshape(2, 128, D).transpose(1, 0, 2)),
        "sb2": f32(np.asarray(inputs["sb2"]).reshape(D, 1)),
        "mw1": f32(mw1),
        "mb1": f32(np.asarray(inputs["mb1"]).reshape(2, 128).T),
        "mw2": f32(mw2.reshape(2, 128, D).transpose(1, 0, 2)),
        "mb2": f32(np.asarray(inputs["mb2"]).reshape(1, D)),
        "ones1": f32(np.ones((1, 128))),
    }

    def sigmoid(x):
        return 1.0 / (1.0 + np.exp(-x))

    def f16(x):
        return np.ascontiguousarray(x, dtype=np.float16)

    seg = cc.reshape(BS, T, N // 512, D)  # [b, t, slice, d]
    mfull0_full = f16(msgs0.transpose(1, 0, 2).reshape(N, EL))
    in_maps = []
    for c in range(NCORES):
        bsl = slice(0, BS)
        sh = slice(c * NT, (c + 1) * NT)
        m = dict(shared)
        m["h0T"] = f32(h0[bsl, sh].transpose(2, 0, 1).reshape(D, R))
        m["hebbT"] = f32(hebb[bsl, sh].transpose(2, 0, 1).reshape(D_ID, R))
        m["identT"] = f32(ident[sh].T)

        injT = np.empty((T, D, NB, NT), dtype=np.float32)
        for q in range(2):
            injT[:, :, :, 512 * q:512 * (q + 1)] = \
                seg[bsl, :, 2 * c + q].transpose(1, 2, 0)[:, :, :, None]
        m["injT"] = f32(injT.reshape(T, D, R))

        # full msgs0, batch-interleaved [n, b, d] (read by step-0 gathers)
        m["mfull0"] = mfull0_full

        # block-diag wrapped sigmoid(w0): blk[32g+k, g, b, c] = s(w0[b,4c+g,k])
        w0 = sigmoid(w_conn0[bsl, sh])          # [NB, NT, K]
        wr = w0.reshape(NB, NT // 4, 4, K)      # [b, c, g, k]
        blk = np.zeros((128, 4, NB, NT // 4), dtype=np.float32)
        for g in range(4):
            blk[32 * g:32 * (g + 1), g] = wr[:, :, g, :].transpose(2, 0, 1)
        m["w0blk"] = f16(blk)

        # gather indices: instr g covers targets 32g..32g+32;
        # lin[i] for i = 1024*g + 128*cp + 32*gp + k  -> conn[4*(8g+cp)+gp, k]
        tgt = conn[sh]                          # [NT, K] global ids
        lin = tgt.reshape(NT // 4, 4, K).reshape(NG, 8, 4, K).reshape(-1)
        wrapped = lin.reshape(2048, 16).T.astype(np.int16)   # [16, 2048]
        m["idx"] = np.ascontiguousarray(np.tile(wrapped, (8, 1)))
        in_maps.append(m)
    return in_maps


def kernel(**inputs) -> np.ndarray:
    key = "prog"
    if key not in _PROGRAM_CACHE:
        _PROGRAM_CACHE[key] = _build_program()
    nc = _PROGRAM_CACHE[key]

    in_maps = _prep_inputs(inputs)
    res = run_bass_kernel_spmd(nc, in_maps, list(range(NCORES)))
    full = np.empty((BS, T, N, D), dtype=np.float32)
    for c in range(NCORES):
        o = np.asarray(res.results[c]["out"]).astype(np.float32)  # [T, D, R]
        o = o.reshape(T, D, NB, NT).transpose(2, 0, 3, 1)
        full[:, :, c * NT:(c + 1) * NT, :] = o
    return full.reshape(BS, T, N // 64, 64 * D)



# revision 25
# speedup vs baseline: 1.0442x; 1.0442x over previous
"""Trainium2 Bass kernel for nn_MemoryGraph (gnn_message_passing).

Sharding: neurons split across 8 cores (1024/core), all 4 batches local.
msgs stored batch-interleaved [n, b, d] in fp32 so ONE 1KB gather
descriptor per edge (n,k) serves all 4 batches; one full-world AllGather
per step (skipped on the last step; step-0 reads a host-built full
buffer, so there is no preamble collective).

Per step (per core, R = 4*1024 rows, cols r = b*1024 + n, feature-major):
  - gather neighbor msg rows from DRAM mfull [8192, 4*64]
  - received = per-target K-weighted sums as tiny PE matmuls
    (stationary = gathered [32k x 64d] block, moving = block-diag w column)
  - MLP matmuls in float32r (1 cycle/row at >=256 free; dst partition 0)
  - mod MLP2 emitted feature-major: ident delta first (per gather phase,
    it gates the state MLP), then 1-decay via replicated-column
    stationary, then w^T wrapped for the next step's tiny-mm movings
  - h kept fp32 in Hf (recurrence noise is amplified ~2x/step, so the
    fed-back state must stay >= f32r precision; bf16/fp16 fail), mirrored
    into f32r C[0:64] for matmuls; output DMA'd as [T, 64, R]
Host side: layout prep in numpy; output reassembly at the end.
"""

import numpy as np
import ml_dtypes

import concourse.bass as bass
import concourse.bacc as bacc
from concourse import mybir, tile, library_config
from concourse.bass_utils import run_bass_kernel_spmd

# problem constants (hardcoded per harness contract)
N, K, D, D_ID = 8192, 32, 64, 32
H = 256
BS, T = 4, 8
NCORES = 8
NB = BS                   # batches per core (all four)
NT = N // NCORES          # 1024 neurons (targets) per core
R = NB * NT               # 4096 rows per core (cols r = b*NT + n_loc)
NG = 32                   # gather instructions per step (1024 idx each)
EL = NB * D               # gathered elem: 256 f32 = 1024B (all batches)

F32 = mybir.dt.float32
F32R = mybir.dt.float32r
BF16 = mybir.dt.bfloat16
F16 = mybir.dt.float16
I16 = mybir.dt.int16
AF = mybir.ActivationFunctionType
ALU = mybir.AluOpType

_PROGRAM_CACHE = {}


def _build_program():
    nc = bacc.Bacc(
        "TRN2", target_bir_lowering=False, debug=False,
        num_devices=NCORES,
    )

    din = {}
    def dram_in(name, shape, dtype=F32):
        din[name] = nc.dram_tensor(name, shape, dtype, kind="ExternalInput")
        return din[name]

    h0T = dram_in("h0T", [D, R], F32R)
    hebbT = dram_in("hebbT", [D_ID, R], F32R)
    identT_in = dram_in("identT", [D_ID, NT])
    injT = dram_in("injT", [T, D, R], F32R)
    mfull0 = dram_in("mfull0", [N, EL], F16)
    w0blk = dram_in("w0blk", [128, 4, NB, NT // 4], F16)
    idx_in = dram_in("idx", [128, 2048], I16)
    dw1C = dram_in("dw1C", [128, H], F32R)
    dw1B = dram_in("dw1B", [128, H], F32R)
    db1 = dram_in("db1", [128, 2])
    dw2wT = dram_in("dw2wT", [128, 2, K], F32R)
    db2wT = dram_in("db2wT", [K, 1])
    dw2om = dram_in("dw2om", [128, 2, D], F32R)
    db2om = dram_in("db2om", [D, 1])
    dw2de = dram_in("dw2de", [128, 2, D_ID], F32R)
    db2de = dram_in("db2de", [D_ID, 1])  # pre-scaled x2 for the 2-core AG
    sw1B = dram_in("sw1B", [128, H], F32R)
    sw1C = dram_in("sw1C", [96, H], F32R)
    sb1 = dram_in("sb1", [128, 2])
    sw2 = dram_in("sw2", [128, 2, D], F32R)
    sb2 = dram_in("sb2", [D, 1])
    mw1 = dram_in("mw1", [96, H], F32R)
    mb1 = dram_in("mb1", [128, 2])
    mw2 = dram_in("mw2", [128, 2, D], F32R)
    mb2 = dram_in("mb2", [1, D], F32R)
    ones1 = dram_in("ones1", [1, 128], F32R)

    out_d = nc.dram_tensor("out", [T, D, R], F32R, kind="ExternalOutput")

    rg_msgs = [list(range(NCORES))]

    with tile.TileContext(nc) as tc:
        with (
            tc.tile_pool(name="persist", bufs=1) as pp,
            tc.tile_pool(name="dram", bufs=1, space="DRAM") as dp,
            tc.tile_pool(name="gpool", bufs=2) as gp,
            tc.tile_pool(name="hidA", bufs=1) as hpA,
            tc.tile_pool(name="hidB", bufs=1) as hpB,
            tc.tile_pool(name="psR", bufs=4, space="PSUM") as psRp,
            tc.tile_pool(name="ps1", bufs=4, space="PSUM") as ps1p,
        ):
            mshard = dp.tile([NT, EL], F16, name="mshard", tag="mshard")
            mfull = dp.tile([N, EL], F16, name="mfull", tag="mfull")

            # persistent SBUF
            C = pp.tile([128, R], F32R)        # [h(64); ide(32); hebb(32)]
            B = pp.tile([128, R], F32R)        # [received(64); inject(64)]
            wTblk = pp.tile([128, 4, NB, NT // 4], F16)   # block-diag w^T
            identM = pp.tile([D_ID, NT], F32)
            omR = pp.tile([D, R], F32)        # (1-decay) replicated over d
            Ttanh = pp.tile([D, R], F32)
            dI = pp.tile([D_ID, NT], F32)     # summed ident delta
            msgs = pp.tile([128, NT // 128, EL], F16)
            idxT = pp.tile([128, 2048], I16)
            onesK = pp.tile([1, 128], F32R)
            onesK_src = onesK  # DMA'd from ones1 input
            # weights
            t_dw1C = pp.tile([128, H], F32R)
            t_dw1B = pp.tile([128, H], F32R)
            t_db1 = pp.tile([128, 2], F32)
            t_dw2wT = pp.tile([128, 2, K], F32R)
            t_db2wT = pp.tile([K, 1], F32)
            t_dw2om = pp.tile([128, 2, D], F32R)
            t_db2om = pp.tile([D, 1], F32)
            t_dw2de = pp.tile([128, 2, D_ID], F32R)
            t_db2de = pp.tile([D_ID, 1], F32)
            t_sw1B = pp.tile([128, H], F32R)
            t_sw1C = pp.tile([96, H], F32R)
            t_sb1 = pp.tile([128, 2], F32)
            t_sw2 = pp.tile([128, 2, D], F32R)
            t_sb2 = pp.tile([D, 1], F32)
            t_mw1 = pp.tile([96, H], F32R)
            t_mb1 = pp.tile([128, 2], F32)
            t_mw2 = pp.tile([128, 2, D], F32R)
            t_mb2 = pp.tile([1, D], F32R)

            # ---------------- preamble ----------------
            nc.gpsimd.load_library(library_config.mlp)

            for tname, ttile in [
                ("dw1C", t_dw1C), ("dw1B", t_dw1B), ("db1", t_db1),
                ("dw2wT", t_dw2wT), ("db2wT", t_db2wT),
                ("dw2om", t_dw2om), ("db2om", t_db2om),
                ("dw2de", t_dw2de), ("db2de", t_db2de),
                ("sw1B", t_sw1B), ("sw1C", t_sw1C), ("sb1", t_sb1),
                ("sw2", t_sw2), ("sb2", t_sb2),
                ("mw1", t_mw1), ("mb1", t_mb1),
                ("mw2", t_mw2), ("mb2", t_mb2), ("ones1", onesK_src),
            ]:
                nc.sync.dma_start(out=ttile[:], in_=din[tname][:])

            nc.sync.dma_start(out=C[0:D, :], in_=h0T[:])
            nc.sync.dma_start(out=C[96:128, :], in_=hebbT[:])
            nc.sync.dma_start(out=identM[:], in_=identT_in[:])
            nc.sync.dma_start(out=wTblk[:], in_=w0blk[:])
            nc.sync.dma_start(out=idxT[:], in_=idx_in[:])
            ide_b = identM[:].unsqueeze(1).broadcast_to([D_ID, NB, NT])
            nc.scalar.copy(
                out=C[D:96, :].rearrange("p (b n) -> p b n", b=NB),
                in_=ide_b)
            # ---------------- time loop ----------------
            for t in range(T):
                nc.sync.dma_start(out=B[D:2 * D, :], in_=injT[t])

                # ---- received: gather + tiny weighted matmuls ----
                modH = hpA.tile([128, 2, R], F32R, tag="hidA")
                for ph in range(2):
                    pR = [psRp.tile([64, 512], F32, tag="psR",
                                    name=f"pR{ph}_{_b}")
                          for _b in range(NB)]
                    for gi in range(16):
                        g = 16 * ph + gi
                        G = gp.tile([128, 8, EL], F16, tag="G")
                        nc.gpsimd.dma_gather(
                            out_ap=G[:],
                            in_ap=(mfull0[:] if t == 0 else mfull[:]),
                            idxs_ap=idxT[:, 64 * g:64 * (g + 1)],
                            num_idxs=1024,
                            num_idxs_reg=1024,
                            elem_size=EL,
                        )
                        for cp in range(8):
                            col = 4 * (8 * gi + cp)
                            c_glob = 8 * g + cp
                            for b in range(NB):
                                nc.tensor.matmul(
                                    pR[b][:, col:col + 4],
                                    G[:, cp, D * b:D * (b + 1)],
                                    wTblk[:, :, b, c_glob],
                                    start=True, stop=True,
                                    skip_group_check=True)
                    # drain received -> B rows 0:64 (b-major cols), on DVE
                    # to keep the Activation queue free for MLP work
                    for b in range(NB):
                        nc.vector.tensor_scalar(
                            out=B[0:D, NT * b + 512 * ph:
                                  NT * b + 512 * (ph + 1)],
                            in0=pR[b][:], scalar1=0.0, scalar2=None,
                            op0=ALU.add)
                    # mod MLP1 for this phase's drained chunks
                    for b in range(NB):
                        sl = slice(NT * b + 512 * ph,
                                   NT * b + 512 * (ph + 1))
                        for m in range(2):
                            ps = ps1p.tile([128, 512], F32, tag="mm1")
                            nc.tensor.matmul(
                                ps[:], t_dw1C[:, 128 * m:128 * (m + 1)],
                                C[:, sl], start=True, stop=False)
                            nc.tensor.matmul(
                                ps[:], t_dw1B[:, 128 * m:128 * (m + 1)],
                                B[:, sl], start=False, stop=True)
                            nc.scalar.activation(
                                out=modH[:, m, sl], in_=ps[:], func=AF.Silu,
                                bias=t_db1[:, m:m + 1])
                    # ident delta for this phase's columns, accumulated on
                    # DVE straight from PSUM (keeps the ACT queue clear)
                    for b in range(NB):
                        dsl = slice(NT * b + 512 * ph,
                                    NT * b + 512 * (ph + 1))
                        csl = slice(512 * ph, 512 * (ph + 1))
                        ps = ps1p.tile([128, 512], F32, tag="mm1")
                        for m in range(2):
                            nc.tensor.matmul(
                                ps[0:D_ID, :], t_dw2de[:, m, :],
                                modH[:, m, dsl],
                                start=(m == 0), stop=(m == 1),
                                skip_group_check=True)
                        if b == 0:
                            nc.vector.tensor_scalar(
                                out=dI[:, csl], in0=ps[0:D_ID, :],
                                scalar1=t_db2de[:], scalar2=None,
                                op0=ALU.add)
                        else:
                            nc.vector.tensor_tensor(
                                out=dI[:, csl], in0=dI[:, csl],
                                in1=ps[0:D_ID, :], op=ALU.add)

                # ---- mod MLP2 (feature-major outputs) ----
                modHr = modH[:].rearrange("p m (b c g) -> p m b c g",
                                          b=NB, g=4)
                # om = 1 - decay, replicated over 64 partitions via
                # column-replicated stationary
                for q in range(8):
                    sl = slice(512 * q, 512 * (q + 1))
                    ps = ps1p.tile([128, 512], F32, tag="mm1")
                    for m in range(2):
                        nc.tensor.matmul(
                            ps[0:D, :], t_dw2om[:, m, :],
                            modH[:, m, sl],
                            start=(m == 0), stop=(m == 1),
                            skip_group_check=True)
                    nc.scalar.activation(
                        out=omR[:, sl], in_=ps[0:D, :], func=AF.Sigmoid,
                        scale=-1.0, bias=t_db2om[:])
                # ---- ident update + broadcast to C rows 64:96 ----
                nc.vector.scalar_tensor_tensor(
                    out=identM[:], in0=dI[:], scalar=1.0 / BS,
                    in1=identM[:], op0=ALU.mult, op1=ALU.add)
                ide_b2 = identM[:].unsqueeze(1).broadcast_to([D_ID, NB, NT])
                nc.scalar.copy(
                    out=C[D:96, :].rearrange("p (b n) -> p b n", b=NB),
                    in_=ide_b2)

                # ---- state / h / msg, pipelined by neuron halves ----
                # half nh covers neurons 512*nh..512*(nh+1) of every batch
                stateH = hpB.tile([128, 2, R], F32R, tag="hidB")
                for nh in range(2):
                    for b in range(NB):
                        sl = slice(NT * b + 512 * nh,
                                   NT * b + 512 * (nh + 1))
                        for m in range(2):
                            ps = ps1p.tile([128, 512], F32, tag="mm1")
                            nc.tensor.matmul(
                                ps[:], t_sw1B[:, 128 * m:128 * (m + 1)],
                                B[:, sl], start=True, stop=False)
                            nc.tensor.matmul(
                                ps[:], t_sw1C[:, 128 * m:128 * (m + 1)],
                                C[0:96, sl], start=False, stop=True)
                            nc.scalar.activation(
                                out=stateH[:, m, sl], in_=ps[:],
                                func=AF.Silu, bias=t_sb1[:, m:m + 1])
                    for b in range(NB):
                        sl = slice(NT * b + 512 * nh,
                                   NT * b + 512 * (nh + 1))
                        ps = ps1p.tile([128, 512], F32, tag="mm1")
                        for m in range(2):
                            nc.tensor.matmul(
                                ps[0:D, :], t_sw2[:, m, :],
                                stateH[:, m, sl],
                                start=(m == 0), stop=(m == 1),
                                skip_group_check=True)
                        nc.scalar.activation(
                            out=Ttanh[:, sl], in_=ps[0:D, :], func=AF.Tanh,
                            bias=t_sb2[:])

                if t < T - 1:
                    msgH = hpB.tile([128, 2, R], F32R, tag="hidB")
                Tv = Ttanh[:].rearrange("p (b n) -> p b n", b=NB)
                Ov = omR[:].rearrange("p (b n) -> p b n", b=NB)
                Cv = C[0:D, :].rearrange("p (b n) -> p b n", b=NB)
                for nh in range(2):
                    s = slice(512 * nh, 512 * (nh + 1))
                    e1, e2 = ((nc.gpsimd, nc.vector) if nh == 0
                              else (nc.vector, nc.gpsimd))
                    # h_new = h + om*(tanh - h), in place in C rows 0:64
                    e1.tensor_tensor(out=Tv[:, :, s], in0=Tv[:, :, s],
                                     in1=Cv[:, :, s], op=ALU.subtract)
                    e1.tensor_tensor(out=Tv[:, :, s], in0=Tv[:, :, s],
                                     in1=Ov[:, :, s], op=ALU.mult)
                    e2.tensor_tensor(out=Cv[:, :, s], in0=Cv[:, :, s],
                                     in1=Tv[:, :, s], op=ALU.add)
                    if t == T - 1:
                        continue
                    # msg MLP for this half, while the other half's
                    # state/h work occupies the remaining engines
                    for b in range(NB):
                        sl = slice(NT * b + 512 * nh,
                                   NT * b + 512 * (nh + 1))
                        for m in range(2):
                            ps = ps1p.tile([128, 512], F32, tag="mm1")
                            nc.tensor.matmul(
                                ps[:], t_mw1[:, 128 * m:128 * (m + 1)],
                                C[0:96, sl], start=True, stop=True)
                            nc.scalar.activation(
                                out=msgH[:, m, sl], in_=ps[:],
                                func=AF.Silu, bias=t_mb1[:, m:m + 1])
                    for j in range(4 * nh, 4 * (nh + 1)):
                        ps = ps1p.tile([128, 512], F32, tag="mm1")
                        for b in range(NB):
                            rsl = slice(NT * b + 128 * j,
                                        NT * b + 128 * (j + 1))
                            nc.tensor.matmul(
                                ps[:, D * b:D * (b + 1)], msgH[:, 0, rsl],
                                t_mw2[:, 0, :], start=True, stop=False,
                                skip_group_check=True)
                            nc.tensor.matmul(
                                ps[:, D * b:D * (b + 1)], msgH[:, 1, rsl],
                                t_mw2[:, 1, :], start=False, stop=False,
                                skip_group_check=True)
                            nc.tensor.matmul(
                                ps[:, D * b:D * (b + 1)], onesK[:],
                                t_mb2[:], start=False, stop=True,
                                skip_group_check=True)
                        nc.scalar.activation(
                            out=msgs[:, j, :], in_=ps[:, 0:EL], func=AF.Tanh)
                    # this half's msgs rows -> DRAM shard piece
                    nc.sync.dma_start(
                        out=mshard[512 * nh:512 * (nh + 1), :]
                        .rearrange("(j p) d -> p j d", p=128),
                        in_=msgs[:, 4 * nh:4 * (nh + 1), :])

                # output word_states for this step (feature-major)
                nc.sync.dma_start(out=out_d[t], in_=C[0:D, :])
                if t == T - 1:
                    continue
                nc.gpsimd.collective_compute(
                    "AllGather", ALU.bypass, ins=[mshard.opt()],
                    outs=[mfull.opt()], replica_groups=rg_msgs)
                # w^T wrap for the NEXT step's received matmuls; modH is
                # kept alive in its own buffer, so this runs inside the
                # AllGather window instead of on the pre-AG critical path.
                # wTblk[32g+k, g, b, c] = sig(w[b, 4c+g, k])
                for g in range(4):
                    for ch in range(2):
                        ps = ps1p.tile([128, 512], F32, tag="mm1")
                        for bi in range(2):
                            b = 2 * ch + bi
                            for m in range(2):
                                nc.tensor.matmul(
                                    ps[0:K, 256 * bi:256 * (bi + 1)],
                                    t_dw2wT[:, m, :],
                                    modHr[:, m, b, :, g],
                                    start=(m == 0), stop=(m == 1),
                                    skip_group_check=True)
                        nc.scalar.activation(
                            out=wTblk[32 * g:32 * (g + 1), g,
                                      2 * ch:2 * (ch + 1), :],
                            in_=ps[0:K, :].rearrange("p (b c) -> p b c", b=2),
                            func=AF.Sigmoid, bias=t_db2wT[:])

    nc.finalize()
    return nc


def _prep_inputs(inputs):
    """Build the per-core input maps from the full problem inputs."""
    cc = np.asarray(inputs["cc_signals"], dtype=np.float32)
    h0 = np.asarray(inputs["h0"], dtype=np.float32)
    msgs0 = np.asarray(inputs["msgs0"], dtype=np.float32)
    w_conn0 = np.asarray(inputs["w_conn0"], dtype=np.float32)
    hebb = np.asarray(inputs["hebbian"], dtype=np.float32)
    ident = np.asarray(inputs["identity"], dtype=np.float32)
    conn = np.asarray(inputs["conn_indices"]).astype(np.int64)

    def f32(x):
        return np.ascontiguousarray(x, dtype=np.float32)

    def bf16(x):
        return np.ascontiguousarray(
            np.asarray(x, dtype=np.float32).astype(ml_dtypes.bfloat16))

    dw1 = np.asarray(inputs["dw1"], dtype=np.float32)   # [256, 256]
    dw2 = np.asarray(inputs["dw2"], dtype=np.float32)   # [256, 65]
    db2 = np.asarray(inputs["db2"], dtype=np.float32)   # [65]
    sw1 = np.asarray(inputs["sw1"], dtype=np.float32)   # [224, 256]
    sw2 = np.asarray(inputs["sw2"], dtype=np.float32)   # [256, 64]
    mw1 = np.asarray(inputs["mw1"], dtype=np.float32)   # [96, 256]
    mw2 = np.asarray(inputs["mw2"], dtype=np.float32)   # [256, 64]

    # dw1 input order: [hebb(0:32), h(32:96), ide(96:128), rcv, inj]
    # C rows: [h, ide, hebb]; B rows: [rcv, inj]
    shared = {
        "dw1C": f32(np.concatenate([dw1[32:96], dw1[96:128], dw1[0:32]])),
        "dw1B": f32(dw1[128:256]),
        "db1": f32(np.asarray(inputs["db1"]).reshape(2, 128).T),
        "dw2wT": f32(dw2[:, 0:K].reshape(2, 128, K).transpose(1, 0, 2)),
        "db2wT": f32(db2[0:K].reshape(K, 1)),
        "dw2om": f32(np.repeat(dw2[:, K:K + 1], D, axis=1)
                      .reshape(2, 128, D).transpose(1, 0, 2)),
        "db2om": f32(np.full((D, 1), -db2[K])),
        "dw2de": f32(dw2[:, K + 1:].reshape(2, 128, D_ID).transpose(1, 0, 2)),
        "db2de": f32(4.0 * db2[K + 1:].reshape(D_ID, 1)),
        "sw1B": f32(sw1[0:128]),
        "sw1C": f32(sw1[128:224]),
        "sb1": f32(np.asarray(inputs["sb1"]).reshape(2, 128).T),
        "sw2": f32(sw2.reshape(2, 128, D).transpose(1, 0, 2)),
        "sb2": f32(np.asarray(inputs["sb2"]).reshape(D, 1)),
        "mw1": f32(mw1),
        "mb1": f32(np.asarray(inputs["mb1"]).reshape(2, 128).T),
        "mw2": f32(mw2.reshape(2, 128, D).transpose(1, 0, 2)),
        "mb2": f32(np.asarray(inputs["mb2"]).reshape(1, D)),
        "ones1": f32(np.ones((1, 128))),
    }

    def sigmoid(x):
        return 1.0 / (1.0 + np.exp(-x))

    def f16(x):
        return np.ascontiguousarray(x, dtype=np.float16)

    seg = cc.reshape(BS, T, N // 512, D)  # [b, t, slice, d]
    mfull0_full = f16(msgs0.transpose(1, 0, 2).reshape(N, EL))
    in_maps = []
    for c in range(NCORES):
        bsl = slice(0, BS)
        sh = slice(c * NT, (c + 1) * NT)
        m = dict(shared)
        m["h0T"] = f32(h0[bsl, sh].transpose(2, 0, 1).reshape(D, R))
        m["hebbT"] = f32(hebb[bsl, sh].transpose(2, 0, 1).reshape(D_ID, R))
        m["identT"] = f32(ident[sh].T)

        injT = np.empty((T, D, NB, NT), dtype=np.float32)
        for q in range(2):
            injT[:, :, :, 512 * q:512 * (q + 1)] = \
                seg[bsl, :, 2 * c + q].transpose(1, 2, 0)[:, :, :, None]
        m["injT"] = f32(injT.reshape(T, D, R))

        # full msgs0, batch-interleaved [n, b, d] (read by step-0 gathers)
        m["mfull0"] = mfull0_full

        # block-diag wrapped sigmoid(w0): blk[32g+k, g, b, c] = s(w0[b,4c+g,k])
        w0 = sigmoid(w_conn0[bsl, sh])          # [NB, NT, K]
        wr = w0.reshape(NB, NT // 4, 4, K)      # [b, c, g, k]
        blk = np.zeros((128, 4, NB, NT // 4), dtype=np.float32)
        for g in range(4):
            blk[32 * g:32 * (g + 1), g] = wr[:, :, g, :].transpose(2, 0, 1)
        m["w0blk"] = f16(blk)

        # gather indices: instr g covers targets 32g..32g+32;
        # lin[i] for i = 1024*g + 128*cp + 32*gp + k  -> conn[4*(8g+cp)+gp, k]
        tgt = conn[sh]                          # [NT, K] global ids
        lin = tgt.reshape(NT // 4, 4, K).reshape(NG, 8, 4, K).reshape(-1)
        wrapped = lin.reshape(2048, 16).T.astype(np.int16)   # [16, 2048]
        m["idx"] = np.ascontiguousarray(np.tile(wrapped, (8, 1)))
        in_maps.append(m)
    return in_maps


def kernel(**inputs) -> np.ndarray:
    key = "prog"
    if key not in _PROGRAM_CACHE:
        _PROGRAM_CACHE[key] = _build_program()
    nc = _PROGRAM_CACHE[key]

    in_maps = _prep_inputs(inputs)
    res = run_bass_kernel_spmd(nc, in_maps, list(range(NCORES)))
    full = np.empty((BS, T, N, D), dtype=np.float32)
    for c in range(NCORES):
        o = np.asarray(res.results[c]["out"]).astype(np.float32)  # [T, D, R]
        o = o.reshape(T, D, NB, NT).transpose(2, 0, 3, 1)
        full[:, :, c * NT:(c + 1) * NT, :] = o
    return full.reshape(BS, T, N // 64, 64 * D)



# revision 28
# speedup vs baseline: 1.0657x; 1.0206x over previous
"""Trainium2 Bass kernel for nn_MemoryGraph (gnn_message_passing).

Sharding: neurons split across 8 cores (1024/core), all 4 batches local.
msgs stored batch-interleaved [n, b, d] in fp32 so ONE 1KB gather
descriptor per edge (n,k) serves all 4 batches; one full-world AllGather
per step (skipped on the last step; step-0 reads a host-built full
buffer, so there is no preamble collective).

Per step (per core, R = 4*1024 rows, cols r = b*1024 + n, feature-major):
  - gather neighbor msg rows from DRAM mfull [8192, 4*64]
  - received = per-target K-weighted sums as tiny PE matmuls
    (stationary = gathered [32k x 64d] block, moving = block-diag w column)
  - MLP matmuls in float32r (1 cycle/row at >=256 free; dst partition 0)
  - mod MLP2 emitted feature-major: ident delta first (per gather phase,
    it gates the state MLP), then 1-decay via replicated-column
    stationary, then w^T wrapped for the next step's tiny-mm movings
  - h kept fp32 in Hf (recurrence noise is amplified ~2x/step, so the
    fed-back state must stay >= f32r precision; bf16/fp16 fail), mirrored
    into f32r C[0:64] for matmuls; output DMA'd as [T, 64, R]
Host side: layout prep in numpy; output reassembly at the end.
"""

import numpy as np
import ml_dtypes

import concourse.bass as bass
import concourse.bacc as bacc
from concourse import mybir, tile, library_config
from concourse.bass_utils import run_bass_kernel_spmd

# problem constants (hardcoded per harness contract)
N, K, D, D_ID = 8192, 32, 64, 32
H = 256
BS, T = 4, 8
NCORES = 8
NB = BS                   # batches per core (all four)
NT = N // NCORES          # 1024 neurons (targets) per core
R = NB * NT               # 4096 rows per core (cols r = b*NT + n_loc)
NG = 32                   # gather instructions per step (1024 idx each)
EL = NB * D               # gathered elem: 256 f32 = 1024B (all batches)

F32 = mybir.dt.float32
F32R = mybir.dt.float32r
BF16 = mybir.dt.bfloat16
F16 = mybir.dt.float16
I16 = mybir.dt.int16
AF = mybir.ActivationFunctionType
ALU = mybir.AluOpType

_PROGRAM_CACHE = {}


def _build_program():
    nc = bacc.Bacc(
        "TRN2", target_bir_lowering=False, debug=False,
        num_devices=NCORES,
    )

    din = {}
    def dram_in(name, shape, dtype=F32):
        din[name] = nc.dram_tensor(name, shape, dtype, kind="ExternalInput")
        return din[name]

    h0T = dram_in("h0T", [D, R], F32R)
    hebbT = dram_in("hebbT", [D_ID, R], F32R)
    identT_in = dram_in("identT", [D_ID, NT])
    injT = dram_in("injT", [T, D, R], F32R)
    mfull0 = dram_in("mfull0", [N, EL], F16)
    w0blk = dram_in("w0blk", [128, 4, NB, NT // 4], F16)
    idx_in = dram_in("idx", [128, 2048], I16)
    dw1C = dram_in("dw1C", [128, H], F32R)
    dw1B = dram_in("dw1B", [128, H], F32R)
    db1 = dram_in("db1", [128, 2])
    dw2wT = dram_in("dw2wT", [128, 2, K], F32R)
    db2wT = dram_in("db2wT", [K, 1])
    dw2om = dram_in("dw2om", [128, 2, D], F32R)
    db2om = dram_in("db2om", [D, 1])
    dw2de = dram_in("dw2de", [128, 2, D_ID], F32R)
    db2de = dram_in("db2de", [D_ID, 1])  # pre-scaled x2 for the 2-core AG
    sw1B = dram_in("sw1B", [128, H], F32R)
    sw1C = dram_in("sw1C", [96, H], F32R)
    sb1 = dram_in("sb1", [128, 2])
    sw2 = dram_in("sw2", [128, 2, D], F32R)
    sb2 = dram_in("sb2", [D, 1])
    mw1 = dram_in("mw1", [96, H], F32R)
    mb1 = dram_in("mb1", [128, 2])
    mw2 = dram_in("mw2", [128, 2, D], F32R)
    mb2 = dram_in("mb2", [1, D], F32R)
    ones1 = dram_in("ones1", [1, 128], F32R)

    out_d = nc.dram_tensor("out", [T, D, R], F32R, kind="ExternalOutput")

    rg_msgs = [list(range(NCORES))]

    with tile.TileContext(nc) as tc:
        with (
            tc.tile_pool(name="persist", bufs=1) as pp,
            tc.tile_pool(name="dram", bufs=1, space="DRAM") as dp,
            tc.tile_pool(name="gpool", bufs=2) as gp,
            tc.tile_pool(name="hidA", bufs=1) as hpA,
            tc.tile_pool(name="hidB", bufs=1) as hpB,
            tc.tile_pool(name="psR", bufs=4, space="PSUM") as psRp,
            tc.tile_pool(name="ps1", bufs=4, space="PSUM") as ps1p,
        ):
            mshard = dp.tile([NT, EL], F16, name="mshard", tag="mshard")
            mfull = dp.tile([N, EL], F16, name="mfull", tag="mfull")

            # persistent SBUF
            C = pp.tile([128, R], F32R)        # [h(64); ide(32); hebb(32)]
            B = pp.tile([128, R], F32R)        # [received(64); inject(64)]
            wTblk = pp.tile([128, 4, NB, NT // 4], F16)   # block-diag w^T
            identM = pp.tile([D_ID, NT], F32)
            omR = pp.tile([D, R], F32)        # (1-decay) replicated over d
            Ttanh = pp.tile([D, R], F32)
            dI = pp.tile([D_ID, NT], F32)     # summed ident delta
            msgs = pp.tile([128, NT // 128, EL], F16)
            idxT = pp.tile([128, 2048], I16)
            onesK = pp.tile([1, 128], F32R)
            onesK_src = onesK  # DMA'd from ones1 input
            # weights
            t_dw1C = pp.tile([128, H], F32R)
            t_dw1B = pp.tile([128, H], F32R)
            t_db1 = pp.tile([128, 2], F32)
            t_dw2wT = pp.tile([128, 2, K], F32R)
            t_db2wT = pp.tile([K, 1], F32)
            t_dw2om = pp.tile([128, 2, D], F32R)
            t_db2om = pp.tile([D, 1], F32)
            t_dw2de = pp.tile([128, 2, D_ID], F32R)
            t_db2de = pp.tile([D_ID, 1], F32)
            t_sw1B = pp.tile([128, H], F32R)
            t_sw1C = pp.tile([96, H], F32R)
            t_sb1 = pp.tile([128, 2], F32)
            t_sw2 = pp.tile([128, 2, D], F32R)
            t_sb2 = pp.tile([D, 1], F32)
            t_mw1 = pp.tile([96, H], F32R)
            t_mb1 = pp.tile([128, 2], F32)
            t_mw2 = pp.tile([128, 2, D], F32R)
            t_mb2 = pp.tile([1, D], F32R)

            # ---------------- preamble ----------------
            nc.gpsimd.load_library(library_config.mlp)

            for tname, ttile in [
                ("dw1C", t_dw1C), ("dw1B", t_dw1B), ("db1", t_db1),
                ("dw2wT", t_dw2wT), ("db2wT", t_db2wT),
                ("dw2om", t_dw2om), ("db2om", t_db2om),
                ("dw2de", t_dw2de), ("db2de", t_db2de),
                ("sw1B", t_sw1B), ("sw1C", t_sw1C), ("sb1", t_sb1),
                ("sw2", t_sw2), ("sb2", t_sb2),
                ("mw1", t_mw1), ("mb1", t_mb1),
                ("mw2", t_mw2), ("mb2", t_mb2), ("ones1", onesK_src),
            ]:
                nc.sync.dma_start(out=ttile[:], in_=din[tname][:])

            nc.sync.dma_start(out=C[0:D, :], in_=h0T[:])
            nc.sync.dma_start(out=C[96:128, :], in_=hebbT[:])
            nc.sync.dma_start(out=identM[:], in_=identT_in[:])
            nc.sync.dma_start(out=wTblk[:], in_=w0blk[:])
            nc.sync.dma_start(out=idxT[:], in_=idx_in[:])
            ide_b = identM[:].unsqueeze(1).broadcast_to([D_ID, NB, NT])
            nc.scalar.copy(
                out=C[D:96, :].rearrange("p (b n) -> p b n", b=NB),
                in_=ide_b)
            # ---------------- time loop ----------------
            for t in range(T):
                nc.sync.dma_start(out=B[D:2 * D, :], in_=injT[t])

                # ---- received: gather + tiny weighted matmuls ----
                modH = hpA.tile([128, 2, R], F32R, tag="hidA")
                for ph in range(2):
                    pR = [psRp.tile([64, 512], F32, tag="psR",
                                    name=f"pR{ph}_{_b}")
                          for _b in range(NB)]
                    for gi in range(16):
                        g = 16 * ph + gi
                        G = gp.tile([128, 8, EL], F16, tag="G")
                        nc.gpsimd.dma_gather(
                            out_ap=G[:],
                            in_ap=(mfull0[:] if t == 0 else mfull[:]),
                            idxs_ap=idxT[:, 64 * g:64 * (g + 1)],
                            num_idxs=1024,
                            num_idxs_reg=1024,
                            elem_size=EL,
                        )
                        for cp in range(8):
                            col = 4 * (8 * gi + cp)
                            c_glob = 8 * g + cp
                            for b in range(NB):
                                nc.tensor.matmul(
                                    pR[b][:, col:col + 4],
                                    G[:, cp, D * b:D * (b + 1)],
                                    wTblk[:, :, b, c_glob],
                                    start=True, stop=True,
                                    skip_group_check=True)
                    # drain received -> B rows 0:64 (b-major cols), on DVE
                    # to keep the Activation queue free for MLP work
                    for b in range(NB):
                        nc.vector.tensor_scalar(
                            out=B[0:D, NT * b + 512 * ph:
                                  NT * b + 512 * (ph + 1)],
                            in0=pR[b][:], scalar1=0.0, scalar2=None,
                            op0=ALU.add)
                    # mod MLP1 for this phase's drained chunks
                    for b in range(NB):
                        sl = slice(NT * b + 512 * ph,
                                   NT * b + 512 * (ph + 1))
                        for m in range(2):
                            ps = ps1p.tile([128, 512], F32, tag="mm1")
                            nc.tensor.matmul(
                                ps[:], t_dw1C[:, 128 * m:128 * (m + 1)],
                                C[:, sl], start=True, stop=False)
                            nc.tensor.matmul(
                                ps[:], t_dw1B[:, 128 * m:128 * (m + 1)],
                                B[:, sl], start=False, stop=True)
                            nc.scalar.activation(
                                out=modH[:, m, sl], in_=ps[:], func=AF.Silu,
                                bias=t_db1[:, m:m + 1])
                    # ident delta for this phase's columns, accumulated on
                    # DVE straight from PSUM (keeps the ACT queue clear)
                    for b in range(NB):
                        dsl = slice(NT * b + 512 * ph,
                                    NT * b + 512 * (ph + 1))
                        csl = slice(512 * ph, 512 * (ph + 1))
                        ps = ps1p.tile([128, 512], F32, tag="mm1")
                        for m in range(2):
                            nc.tensor.matmul(
                                ps[0:D_ID, :], t_dw2de[:, m, :],
                                modH[:, m, dsl],
                                start=(m == 0), stop=(m == 1),
                                skip_group_check=True)
                        if b == 0:
                            nc.vector.tensor_scalar(
                                out=dI[:, csl], in0=ps[0:D_ID, :],
                                scalar1=t_db2de[:], scalar2=None,
                                op0=ALU.add)
                        else:
                            nc.vector.tensor_tensor(
                                out=dI[:, csl], in0=dI[:, csl],
                                in1=ps[0:D_ID, :], op=ALU.add)

                # ---- ident update: write ide2 straight into C rows 64:96
                # per batch on DVE (runs under the om sigmoids on ACT) ----
                Cide = C[D:96, :].rearrange("p (b n) -> p b n", b=NB)
                for b in range(NB):
                    nc.vector.scalar_tensor_tensor(
                        out=Cide[:, b, :], in0=dI[:], scalar=1.0 / BS,
                        in1=identM[:], op0=ALU.mult, op1=ALU.add)
                nc.vector.scalar_tensor_tensor(
                    out=identM[:], in0=dI[:], scalar=1.0 / BS,
                    in1=identM[:], op0=ALU.mult, op1=ALU.add)

                # ---- mod MLP2 (feature-major outputs) ----
                modHr = modH[:].rearrange("p m (b c g) -> p m b c g",
                                          b=NB, g=4)
                # om = 1 - decay, replicated over 64 partitions via
                # column-replicated stationary
                for q in range(8):
                    sl = slice(512 * q, 512 * (q + 1))
                    ps = ps1p.tile([128, 512], F32, tag="mm1")
                    for m in range(2):
                        nc.tensor.matmul(
                            ps[0:D, :], t_dw2om[:, m, :],
                            modH[:, m, sl],
                            start=(m == 0), stop=(m == 1),
                            skip_group_check=True)
                    nc.scalar.activation(
                        out=omR[:, sl], in_=ps[0:D, :], func=AF.Sigmoid,
                        scale=-1.0, bias=t_db2om[:])

                # ---- state / h / msg, pipelined by neuron halves ----
                # half nh covers neurons 512*nh..512*(nh+1) of every batch
                stateH = hpB.tile([128, 2, R], F32R, tag="hidB")
                for nh in range(2):
                    for b in range(NB):
                        sl = slice(NT * b + 512 * nh,
                                   NT * b + 512 * (nh + 1))
                        for m in range(2):
                            ps = ps1p.tile([128, 512], F32, tag="mm1")
                            nc.tensor.matmul(
                                ps[:], t_sw1B[:, 128 * m:128 * (m + 1)],
                                B[:, sl], start=True, stop=False)
                            nc.tensor.matmul(
                                ps[:], t_sw1C[:, 128 * m:128 * (m + 1)],
                                C[0:96, sl], start=False, stop=True)
                            nc.scalar.activation(
                                out=stateH[:, m, sl], in_=ps[:],
                                func=AF.Silu, bias=t_sb1[:, m:m + 1])
                    for b in range(NB):
                        sl = slice(NT * b + 512 * nh,
                                   NT * b + 512 * (nh + 1))
                        ps = ps1p.tile([128, 512], F32, tag="mm1")
                        for m in range(2):
                            nc.tensor.matmul(
                                ps[0:D, :], t_sw2[:, m, :],
                                stateH[:, m, sl],
                                start=(m == 0), stop=(m == 1),
                                skip_group_check=True)
                        nc.scalar.activation(
                            out=Ttanh[:, sl], in_=ps[0:D, :], func=AF.Tanh,
                            bias=t_sb2[:])

                if t < T - 1:
                    msgH = hpB.tile([128, 2, R], F32R, tag="hidB")
                Tv = Ttanh[:].rearrange("p (b n) -> p b n", b=NB)
                Ov = omR[:].rearrange("p (b n) -> p b n", b=NB)
                Cv = C[0:D, :].rearrange("p (b n) -> p b n", b=NB)
                for nh in range(2):
                    s = slice(512 * nh, 512 * (nh + 1))
                    # h_new = h + om*(tanh - h), in place in C rows 0:64;
                    # batch-pair chunks alternate Pool/DVE so both engines
                    # pipeline the 3-op chain
                    for bp in range(2):
                        bsl = slice(2 * bp, 2 * bp + 2)
                        e1, e2 = ((nc.gpsimd, nc.vector)
                                  if (nh + bp) % 2 == 0
                                  else (nc.vector, nc.gpsimd))
                        e1.tensor_tensor(out=Tv[:, bsl, s],
                                         in0=Tv[:, bsl, s],
                                         in1=Cv[:, bsl, s],
                                         op=ALU.subtract)
                        e1.tensor_tensor(out=Tv[:, bsl, s],
                                         in0=Tv[:, bsl, s],
                                         in1=Ov[:, bsl, s], op=ALU.mult)
                        e2.tensor_tensor(out=Cv[:, bsl, s],
                                         in0=Cv[:, bsl, s],
                                         in1=Tv[:, bsl, s], op=ALU.add)
                    if t == T - 1:
                        continue
                    # msg MLP for this half, while the other half's
                    # state/h work occupies the remaining engines
                    for b in range(NB):
                        sl = slice(NT * b + 512 * nh,
                                   NT * b + 512 * (nh + 1))
                        for m in range(2):
                            ps = ps1p.tile([128, 512], F32, tag="mm1")
                            nc.tensor.matmul(
                                ps[:], t_mw1[:, 128 * m:128 * (m + 1)],
                                C[0:96, sl], start=True, stop=True)
                            nc.scalar.activation(
                                out=msgH[:, m, sl], in_=ps[:],
                                func=AF.Silu, bias=t_mb1[:, m:m + 1])
                    for j in range(4 * nh, 4 * (nh + 1)):
                        ps = ps1p.tile([128, 512], F32, tag="mm1")
                        for b in range(NB):
                            rsl = slice(NT * b + 128 * j,
                                        NT * b + 128 * (j + 1))
                            nc.tensor.matmul(
                                ps[:, D * b:D * (b + 1)], msgH[:, 0, rsl],
                                t_mw2[:, 0, :], start=True, stop=False,
                                skip_group_check=True)
                            nc.tensor.matmul(
                                ps[:, D * b:D * (b + 1)], msgH[:, 1, rsl],
                                t_mw2[:, 1, :], start=False, stop=False,
                                skip_group_check=True)
                            nc.tensor.matmul(
                                ps[:, D * b:D * (b + 1)], onesK[:],
                                t_mb2[:], start=False, stop=True,
                                skip_group_check=True)
                        nc.scalar.activation(
                            out=msgs[:, j, :], in_=ps[:, 0:EL], func=AF.Tanh)
                    # this half's msgs rows -> DRAM shard piece
                    nc.sync.dma_start(
                        out=mshard[512 * nh:512 * (nh + 1), :]
                        .rearrange("(j p) d -> p j d", p=128),
                        in_=msgs[:, 4 * nh:4 * (nh + 1), :])

                # output word_states for this step (feature-major)
                nc.sync.dma_start(out=out_d[t], in_=C[0:D, :])
                if t == T - 1:
                    continue
                nc.gpsimd.collective_compute(
                    "AllGather", ALU.bypass, ins=[mshard.opt()],
                    outs=[mfull.opt()], replica_groups=rg_msgs)
                # w^T wrap for the NEXT step's received matmuls; modH is
                # kept alive in its own buffer, so this runs inside the
                # AllGather window instead of on the pre-AG critical path.
                # wTblk[32g+k, g, b, c] = sig(w[b, 4c+g, k])
                for g in range(4):
                    for ch in range(2):
                        ps = ps1p.tile([128, 512], F32, tag="mm1")
                        for bi in range(2):
                            b = 2 * ch + bi
                            for m in range(2):
                                nc.tensor.matmul(
                                    ps[0:K, 256 * bi:256 * (bi + 1)],
                                    t_dw2wT[:, m, :],
                                    modHr[:, m, b, :, g],
                                    start=(m == 0), stop=(m == 1),
                                    skip_group_check=True)
                        nc.scalar.activation(
                            out=wTblk[32 * g:32 * (g + 1), g,
                                      2 * ch:2 * (ch + 1), :],
                            in_=ps[0:K, :].rearrange("p (b c) -> p b c", b=2),
                            func=AF.Sigmoid, bias=t_db2wT[:])

    nc.finalize()
    return nc


def _prep_inputs(inputs):
    """Build the per-core input maps from the full problem inputs."""
    cc = np.asarray(inputs["cc_signals"], dtype=np.float32)
    h0 = np.asarray(inputs["h0"], dtype=np.float32)
    msgs0 = np.asarray(inputs["msgs0"], dtype=np.float32)
    w_conn0 = np.asarray(inputs["w_conn0"], dtype=np.float32)
    hebb = np.asarray(inputs["hebbian"], dtype=np.float32)
    ident = np.asarray(inputs["identity"], dtype=np.float32)
    conn = np.asarray(inputs["conn_indices"]).astype(np.int64)

    def f32(x):
        return np.ascontiguousarray(x, dtype=np.float32)

    def bf16(x):
        return np.ascontiguousarray(
            np.asarray(x, dtype=np.float32).astype(ml_dtypes.bfloat16))

    dw1 = np.asarray(inputs["dw1"], dtype=np.float32)   # [256, 256]
    dw2 = np.asarray(inputs["dw2"], dtype=np.float32)   # [256, 65]
    db2 = np.asarray(inputs["db2"], dtype=np.float32)   # [65]
    sw1 = np.asarray(inputs["sw1"], dtype=np.float32)   # [224, 256]
    sw2 = np.asarray(inputs["sw2"], dtype=np.float32)   # [256, 64]
    mw1 = np.asarray(inputs["mw1"], dtype=np.float32)   # [96, 256]
    mw2 = np.asarray(inputs["mw2"], dtype=np.float32)   # [256, 64]

    # dw1 input order: [hebb(0:32), h(32:96), ide(96:128), rcv, inj]
    # C rows: [h, ide, hebb]; B rows: [rcv, inj]
    shared = {
        "dw1C": f32(np.concatenate([dw1[32:96], dw1[96:128], dw1[0:32]])),
        "dw1B": f32(dw1[128:256]),
        "db1": f32(np.asarray(inputs["db1"]).reshape(2, 128).T),
        "dw2wT": f32(dw2[:, 0:K].reshape(2, 128, K).transpose(1, 0, 2)),
        "db2wT": f32(db2[0:K].reshape(K, 1)),
        "dw2om": f32(np.repeat(dw2[:, K:K + 1], D, axis=1)
                      .reshape(2, 128, D).transpose(1, 0, 2)),
        "db2om": f32(np.full((D, 1), -db2[K])),
        "dw2de": f32(dw2[:, K + 1:].reshape(2, 128, D_ID).transpose(1, 0, 2)),
        "db2de": f32(4.0 * db2[K + 1:].reshape(D_ID, 1)),
        "sw1B": f32(sw1[0:128]),
        "sw1C": f32(sw1[128:224]),
        "sb1": f32(np.asarray(inputs["sb1"]).reshape(2, 128).T),
        "sw2": f32(sw2.reshape(2, 128, D).transpose(1, 0, 2)),
        "sb2": f32(np.asarray(inputs["sb2"]).reshape(D, 1)),
        "mw1": f32(mw1),
        "mb1": f32(np.asarray(inputs["mb1"]).reshape(2, 128).T),
        "mw2": f32(mw2.reshape(2, 128, D).transpose(1, 0, 2)),
        "mb2": f32(np.asarray(inputs["mb2"]).reshape(1, D)),
        "ones1": f32(np.ones((1, 128))),
    }

    def sigmoid(x):
        return 1.0 / (1.0 + np.exp(-x))

    def f16(x):
        return np.ascontiguousarray(x, dtype=np.float16)

    seg = cc.reshape(BS, T, N // 512, D)  # [b, t, slice, d]
    mfull0_full = f16(msgs0.transpose(1, 0, 2).reshape(N, EL))
    in_maps = []
    for c in range(NCORES):
        bsl = slice(0, BS)
        sh = slice(c * NT, (c + 1) * NT)
        m = dict(shared)
        m["h0T"] = f32(h0[bsl, sh].transpose(2, 0, 1).reshape(D, R))
        m["hebbT"] = f32(hebb[bsl, sh].transpose(2, 0, 1).reshape(D_ID, R))
        m["identT"] = f32(ident[sh].T)

        injT = np.empty((T, D, NB, NT), dtype=np.float32)
        for q in range(2):
            injT[:, :, :, 512 * q:512 * (q + 1)] = \
                seg[bsl, :, 2 * c + q].transpose(1, 2, 0)[:, :, :, None]
        m["injT"] = f32(injT.reshape(T, D, R))

        # full msgs0, batch-interleaved [n, b, d] (read by step-0 gathers)
        m["mfull0"] = mfull0_full

        # block-diag wrapped sigmoid(w0): blk[32g+k, g, b, c] = s(w0[b,4c+g,k])
        w0 = sigmoid(w_conn0[bsl, sh])          # [NB, NT, K]
        wr = w0.reshape(NB, NT // 4, 4, K)      # [b, c, g, k]
        blk = np.zeros((128, 4, NB, NT // 4), dtype=np.float32)
        for g in range(4):
            blk[32 * g:32 * (g + 1), g] = wr[:, :, g, :].transpose(2, 0, 1)
        m["w0blk"] = f16(blk)

        # gather indices: instr g covers targets 32g..32g+32;
        # lin[i] for i = 1024*g + 128*cp + 32*gp + k  -> conn[4*(8g+cp)+gp, k]
        tgt = conn[sh]                          # [NT, K] global ids
        lin = tgt.reshape(NT // 4, 4, K).reshape(NG, 8, 4, K).reshape(-1)
        wrapped = lin.reshape(2048, 16).T.astype(np.int16)   # [16, 2048]
        m["idx"] = np.ascontiguousarray(np.tile(wrapped, (8, 1)))
        in_maps.append(m)
    return in_maps


def kernel(**inputs) -> np.ndarray:
    key = "prog"
    if key not in _PROGRAM_CACHE:
        _PROGRAM_CACHE[key] = _build_program()
    nc = _PROGRAM_CACHE[key]

    in_maps = _prep_inputs(inputs)
    res = run_bass_kernel_spmd(nc, in_maps, list(range(NCORES)))
    full = np.empty((BS, T, N, D), dtype=np.float32)
    for c in range(NCORES):
        o = np.asarray(res.results[c]["out"]).astype(np.float32)  # [T, D, R]
        o = o.reshape(T, D, NB, NT).transpose(2, 0, 3, 1)
        full[:, :, c * NT:(c + 1) * NT, :] = o
    return full.reshape(BS, T, N // 64, 64 * D)



# revision 31
# speedup vs baseline: 1.0844x; 1.0175x over previous
"""Trainium2 Bass kernel for nn_MemoryGraph (gnn_message_passing).

Sharding: neurons split across 8 cores (1024/core), all 4 batches local.
msgs stored batch-interleaved [n, b, d] in fp32 so ONE 1KB gather
descriptor per edge (n,k) serves all 4 batches; one full-world AllGather
per step (skipped on the last step; step-0 reads a host-built full
buffer, so there is no preamble collective).

Per step (per core, R = 4*1024 rows, cols r = b*1024 + n, feature-major):
  - gather neighbor msg rows from DRAM mfull [8192, 4*64]
  - received = per-target K-weighted sums as tiny PE matmuls
    (stationary = gathered [32k x 64d] block, moving = block-diag w column)
  - MLP matmuls in float32r (1 cycle/row at >=256 free; dst partition 0)
  - mod MLP2 emitted feature-major: ident delta first (per gather phase,
    it gates the state MLP), then 1-decay via replicated-column
    stationary, then w^T wrapped for the next step's tiny-mm movings
  - h kept fp32 in Hf (recurrence noise is amplified ~2x/step, so the
    fed-back state must stay >= f32r precision; bf16/fp16 fail), mirrored
    into f32r C[0:64] for matmuls; output DMA'd as [T, 64, R]
Host side: layout prep in numpy; output reassembly at the end.
"""

import numpy as np
import ml_dtypes

import concourse.bass as bass
import concourse.bacc as bacc
from concourse import mybir, tile, library_config
from concourse.bass_utils import run_bass_kernel_spmd

# problem constants (hardcoded per harness contract)
N, K, D, D_ID = 8192, 32, 64, 32
H = 256
BS, T = 4, 8
NCORES = 8
NB = BS                   # batches per core (all four)
NT = N // NCORES          # 1024 neurons (targets) per core
R = NB * NT               # 4096 rows per core (cols r = b*NT + n_loc)
NG = 32                   # gather instructions per step (1024 idx each)
EL = NB * D               # gathered elem: 256 f32 = 1024B (all batches)

F32 = mybir.dt.float32
F32R = mybir.dt.float32r
BF16 = mybir.dt.bfloat16
F16 = mybir.dt.float16
I16 = mybir.dt.int16
AF = mybir.ActivationFunctionType
ALU = mybir.AluOpType

_PROGRAM_CACHE = {}


def _build_program():
    nc = bacc.Bacc(
        "TRN2", target_bir_lowering=False, debug=False,
        num_devices=NCORES,
    )

    din = {}
    def dram_in(name, shape, dtype=F32):
        din[name] = nc.dram_tensor(name, shape, dtype, kind="ExternalInput")
        return din[name]

    h0T = dram_in("h0T", [D, R], F32R)
    hebbT = dram_in("hebbT", [D_ID, R], F32R)
    identT_in = dram_in("identT", [D_ID, NT])
    injT = dram_in("injT", [T, D, R], F32R)
    mfull0 = dram_in("mfull0", [N, EL], F16)
    w0blk = dram_in("w0blk", [128, 4, NB, NT // 4], F16)
    idx_in = dram_in("idx", [128, 2048], I16)
    dw1C = dram_in("dw1C", [128, H], F32R)
    dw1B = dram_in("dw1B", [128, H], F32R)
    db1 = dram_in("db1", [128, 2])
    dw2wT = dram_in("dw2wT", [128, 2, K], F32R)
    db2wT = dram_in("db2wT", [K, 1])
    dw2om = dram_in("dw2om", [128, 2, D], F32R)
    db2om = dram_in("db2om", [D, 1])
    dw2de = dram_in("dw2de", [128, 2, D_ID], F32R)
    db2de = dram_in("db2de", [D_ID, 1])  # pre-scaled x2 for the 2-core AG
    sw1B = dram_in("sw1B", [128, H], F32R)
    sw1C = dram_in("sw1C", [96, H], F32R)
    sb1 = dram_in("sb1", [128, 2])
    sw2 = dram_in("sw2", [128, 2, D], F32R)
    sb2 = dram_in("sb2", [D, 1])
    mw1 = dram_in("mw1", [96, H], F32R)
    mb1 = dram_in("mb1", [128, 2])
    mw2 = dram_in("mw2", [128, 2, D], F16)
    mb2 = dram_in("mb2", [1, EL], F16)
    ones1 = dram_in("ones1", [1, 128], F16)

    out_d = nc.dram_tensor("out", [T, D, R], F32R, kind="ExternalOutput")

    rg_msgs = [list(range(NCORES))]

    with tile.TileContext(nc) as tc:
        with (
            tc.tile_pool(name="persist", bufs=1) as pp,
            tc.tile_pool(name="dram", bufs=1, space="DRAM") as dp,
            tc.tile_pool(name="gpool", bufs=2) as gp,
            tc.tile_pool(name="hidA", bufs=1) as hpA,
            tc.tile_pool(name="hidB", bufs=1) as hpB,
            tc.tile_pool(name="psR", bufs=4, space="PSUM") as psRp,
            tc.tile_pool(name="ps1", bufs=4, space="PSUM") as ps1p,
        ):
            mshard = dp.tile([NT, EL], F16, name="mshard", tag="mshard")
            mfull = dp.tile([N, EL], F16, name="mfull", tag="mfull")

            # persistent SBUF
            C = pp.tile([128, R], F32R)        # [h(64); ide(32); hebb(32)]
            B = pp.tile([128, R], F32R)        # [received(64); inject(64)]
            wTblk = pp.tile([128, 4, NB, NT // 4], F16)   # block-diag w^T
            identM = pp.tile([D_ID, NT], F32)
            omR = pp.tile([D, R], F32)        # (1-decay) replicated over d
            Ttanh = pp.tile([D, R], F32)
            dI = pp.tile([D_ID, NT], F32)     # summed ident delta
            msgs = pp.tile([128, NT // 128, EL], F16)
            idxT = pp.tile([128, 2048], I16)
            onesK = pp.tile([1, 128], F16)
            onesK_src = onesK  # DMA'd from ones1 input
            # weights
            t_dw1C = pp.tile([128, H], F32R)
            t_dw1B = pp.tile([128, H], F32R)
            t_db1 = pp.tile([128, 2], F32)
            t_dw2wT = pp.tile([128, 2, K], F32R)
            t_db2wT = pp.tile([K, 1], F32)
            t_dw2om = pp.tile([128, 2, D], F32R)
            t_db2om = pp.tile([D, 1], F32)
            t_dw2de = pp.tile([128, 2, D_ID], F32R)
            t_db2de = pp.tile([D_ID, 1], F32)
            t_sw1B = pp.tile([128, H], F32R)
            t_sw1C = pp.tile([96, H], F32R)
            t_sb1 = pp.tile([128, 2], F32)
            t_sw2 = pp.tile([128, 2, D], F32R)
            t_sb2 = pp.tile([D, 1], F32)
            t_mw1 = pp.tile([96, H], F32R)
            t_mb1 = pp.tile([128, 2], F32)
            t_mw2 = pp.tile([128, 2, D], F16)
            t_mb2 = pp.tile([1, EL], F16)

            # ---------------- preamble ----------------
            nc.gpsimd.load_library(library_config.mlp)

            for tname, ttile in [
                ("dw1C", t_dw1C), ("dw1B", t_dw1B), ("db1", t_db1),
                ("dw2wT", t_dw2wT), ("db2wT", t_db2wT),
                ("dw2om", t_dw2om), ("db2om", t_db2om),
                ("dw2de", t_dw2de), ("db2de", t_db2de),
                ("sw1B", t_sw1B), ("sw1C", t_sw1C), ("sb1", t_sb1),
                ("sw2", t_sw2), ("sb2", t_sb2),
                ("mw1", t_mw1), ("mb1", t_mb1),
                ("mw2", t_mw2), ("mb2", t_mb2), ("ones1", onesK_src),
            ]:
                nc.sync.dma_start(out=ttile[:], in_=din[tname][:])

            nc.sync.dma_start(out=C[0:D, :], in_=h0T[:])
            nc.sync.dma_start(out=C[96:128, :], in_=hebbT[:])
            nc.sync.dma_start(out=identM[:], in_=identT_in[:])
            nc.sync.dma_start(out=wTblk[:], in_=w0blk[:])
            nc.sync.dma_start(out=idxT[:], in_=idx_in[:])
            ide_b = identM[:].unsqueeze(1).broadcast_to([D_ID, NB, NT])
            nc.scalar.copy(
                out=C[D:96, :].rearrange("p (b n) -> p b n", b=NB),
                in_=ide_b)
            # ---------------- time loop ----------------
            for t in range(T):
                nc.sync.dma_start(out=B[D:2 * D, :], in_=injT[t])

                # ---- received: gather + tiny weighted matmuls ----
                modH = hpA.tile([128, 2, R], F32R, tag="hidA")
                for ph in range(2):
                    pR = [psRp.tile([64, 512], F32, tag="psR",
                                    name=f"pR{ph}_{_b}")
                          for _b in range(NB)]
                    for gi in range(16):
                        g = 16 * ph + gi
                        G = gp.tile([128, 8, EL], F16, tag="G")
                        nc.gpsimd.dma_gather(
                            out_ap=G[:],
                            in_ap=(mfull0[:] if t == 0 else mfull[:]),
                            idxs_ap=idxT[:, 64 * g:64 * (g + 1)],
                            num_idxs=1024,
                            num_idxs_reg=1024,
                            elem_size=EL,
                        )
                        for cp in range(8):
                            col = 4 * (8 * gi + cp)
                            c_glob = 8 * g + cp
                            for b in range(NB):
                                nc.tensor.matmul(
                                    pR[b][:, col:col + 4],
                                    G[:, cp, D * b:D * (b + 1)],
                                    wTblk[:, :, b, c_glob],
                                    start=True, stop=True,
                                    skip_group_check=True)
                    # drain received -> B rows 0:64 (b-major cols), on DVE
                    # to keep the Activation queue free for MLP work
                    for b in range(NB):
                        nc.vector.tensor_scalar(
                            out=B[0:D, NT * b + 512 * ph:
                                  NT * b + 512 * (ph + 1)],
                            in0=pR[b][:], scalar1=0.0, scalar2=None,
                            op0=ALU.add)
                    # mod MLP1 for this phase's drained chunks
                    for b in range(NB):
                        sl = slice(NT * b + 512 * ph,
                                   NT * b + 512 * (ph + 1))
                        for m in range(2):
                            ps = ps1p.tile([128, 512], F32, tag="mm1")
                            nc.tensor.matmul(
                                ps[:], t_dw1C[:, 128 * m:128 * (m + 1)],
                                C[:, sl], start=True, stop=False)
                            nc.tensor.matmul(
                                ps[:], t_dw1B[:, 128 * m:128 * (m + 1)],
                                B[:, sl], start=False, stop=True)
                            nc.scalar.activation(
                                out=modH[:, m, sl], in_=ps[:], func=AF.Silu,
                                bias=t_db1[:, m:m + 1])
                    # ident delta for this phase's columns, accumulated on
                    # DVE straight from PSUM (keeps the ACT queue clear)
                    for b in range(NB):
                        dsl = slice(NT * b + 512 * ph,
                                    NT * b + 512 * (ph + 1))
                        csl = slice(512 * ph, 512 * (ph + 1))
                        ps = ps1p.tile([128, 512], F32, tag="mm1")
                        for m in range(2):
                            nc.tensor.matmul(
                                ps[0:D_ID, :], t_dw2de[:, m, :],
                                modH[:, m, dsl],
                                start=(m == 0), stop=(m == 1),
                                skip_group_check=True)
                        if b == 0:
                            nc.vector.tensor_scalar(
                                out=dI[:, csl], in0=ps[0:D_ID, :],
                                scalar1=t_db2de[:], scalar2=None,
                                op0=ALU.add)
                        else:
                            nc.vector.tensor_tensor(
                                out=dI[:, csl], in0=dI[:, csl],
                                in1=ps[0:D_ID, :], op=ALU.add)

                # ---- ident update: write ide2 straight into C rows 64:96
                # per batch on DVE (runs under the om sigmoids on ACT) ----
                Cide = C[D:96, :].rearrange("p (b n) -> p b n", b=NB)
                for b in range(NB):
                    nc.vector.scalar_tensor_tensor(
                        out=Cide[:, b, :], in0=dI[:], scalar=1.0 / BS,
                        in1=identM[:], op0=ALU.mult, op1=ALU.add)
                nc.vector.scalar_tensor_tensor(
                    out=identM[:], in0=dI[:], scalar=1.0 / BS,
                    in1=identM[:], op0=ALU.mult, op1=ALU.add)

                # ---- mod MLP2 (feature-major outputs) ----
                modHr = modH[:].rearrange("p m (b c g) -> p m b c g",
                                          b=NB, g=4)
                # om = 1 - decay, replicated over 64 partitions via
                # column-replicated stationary
                for q in range(8):
                    sl = slice(512 * q, 512 * (q + 1))
                    ps = ps1p.tile([128, 512], F32, tag="mm1")
                    for m in range(2):
                        nc.tensor.matmul(
                            ps[0:D, :], t_dw2om[:, m, :],
                            modH[:, m, sl],
                            start=(m == 0), stop=(m == 1),
                            skip_group_check=True)
                    nc.scalar.activation(
                        out=omR[:, sl], in_=ps[0:D, :], func=AF.Sigmoid,
                        scale=-1.0, bias=t_db2om[:])

                # ---- state / h / msg, pipelined by neuron halves ----
                # half nh covers neurons 512*nh..512*(nh+1) of every batch
                stateH = hpB.tile([128, 2, R], F32R, tag="hidB")
                for nh in range(2):
                    for b in range(NB):
                        sl = slice(NT * b + 512 * nh,
                                   NT * b + 512 * (nh + 1))
                        for m in range(2):
                            ps = ps1p.tile([128, 512], F32, tag="mm1")
                            nc.tensor.matmul(
                                ps[:], t_sw1B[:, 128 * m:128 * (m + 1)],
                                B[:, sl], start=True, stop=False)
                            nc.tensor.matmul(
                                ps[:], t_sw1C[:, 128 * m:128 * (m + 1)],
                                C[0:96, sl], start=False, stop=True)
                            nc.scalar.activation(
                                out=stateH[:, m, sl], in_=ps[:],
                                func=AF.Silu, bias=t_sb1[:, m:m + 1])
                    for b in range(NB):
                        sl = slice(NT * b + 512 * nh,
                                   NT * b + 512 * (nh + 1))
                        ps = ps1p.tile([128, 512], F32, tag="mm1")
                        for m in range(2):
                            nc.tensor.matmul(
                                ps[0:D, :], t_sw2[:, m, :],
                                stateH[:, m, sl],
                                start=(m == 0), stop=(m == 1),
                                skip_group_check=True)
                        nc.scalar.activation(
                            out=Ttanh[:, sl], in_=ps[0:D, :], func=AF.Tanh,
                            bias=t_sb2[:])

                if t < T - 1:
                    msgH = hpB.tile([128, 2, R], F16, tag="hidB")
                Tv = Ttanh[:].rearrange("p (b n) -> p b n", b=NB)
                Ov = omR[:].rearrange("p (b n) -> p b n", b=NB)
                Cv = C[0:D, :].rearrange("p (b n) -> p b n", b=NB)
                for nh in range(2):
                    s = slice(512 * nh, 512 * (nh + 1))
                    # h_new = h + om*(tanh - h), in place in C rows 0:64;
                    # batch-pair chunks alternate Pool/DVE so both engines
                    # pipeline the 3-op chain
                    for bp in range(2):
                        bsl = slice(2 * bp, 2 * bp + 2)
                        e1, e2 = ((nc.gpsimd, nc.vector)
                                  if (nh + bp) % 2 == 0
                                  else (nc.vector, nc.gpsimd))
                        e1.tensor_tensor(out=Tv[:, bsl, s],
                                         in0=Tv[:, bsl, s],
                                         in1=Cv[:, bsl, s],
                                         op=ALU.subtract)
                        e1.tensor_tensor(out=Tv[:, bsl, s],
                                         in0=Tv[:, bsl, s],
                                         in1=Ov[:, bsl, s], op=ALU.mult)
                        e2.tensor_tensor(out=Cv[:, bsl, s],
                                         in0=Cv[:, bsl, s],
                                         in1=Tv[:, bsl, s], op=ALU.add)
                    if t == T - 1:
                        continue
                    # msg MLP for this half, while the other half's
                    # state/h work occupies the remaining engines
                    for b in range(NB):
                        sl = slice(NT * b + 512 * nh,
                                   NT * b + 512 * (nh + 1))
                        for m in range(2):
                            ps = ps1p.tile([128, 512], F32, tag="mm1")
                            nc.tensor.matmul(
                                ps[:], t_mw1[:, 128 * m:128 * (m + 1)],
                                C[0:96, sl], start=True, stop=True)
                            nc.scalar.activation(
                                out=msgH[:, m, sl], in_=ps[:],
                                func=AF.Silu, bias=t_mb1[:, m:m + 1])
                    for j in range(4 * nh, 4 * (nh + 1)):
                        ps = ps1p.tile([128, 512], F32, tag="mm1")
                        for b in range(NB):
                            rsl = slice(NT * b + 128 * j,
                                        NT * b + 128 * (j + 1))
                            nc.tensor.matmul(
                                ps[:, D * b:D * (b + 1)], msgH[:, 0, rsl],
                                t_mw2[:, 0, :], start=True, stop=False,
                                skip_group_check=True)
                            nc.tensor.matmul(
                                ps[:, D * b:D * (b + 1)], msgH[:, 1, rsl],
                                t_mw2[:, 1, :], start=False, stop=False,
                                skip_group_check=True)
                            nc.tensor.matmul(
                                ps[:, D * b:D * (b + 1)], onesK[:],
                                t_mb2[:, D * b:D * (b + 1)],
                                start=False, stop=True,
                                skip_group_check=True)
                        nc.scalar.activation(
                            out=msgs[:, j, :], in_=ps[:, 0:EL], func=AF.Tanh)
                    # this half's msgs rows -> DRAM shard piece
                    nc.sync.dma_start(
                        out=mshard[512 * nh:512 * (nh + 1), :]
                        .rearrange("(j p) d -> p j d", p=128),
                        in_=msgs[:, 4 * nh:4 * (nh + 1), :])

                # output word_states for this step (feature-major)
                nc.sync.dma_start(out=out_d[t], in_=C[0:D, :])
                if t == T - 1:
                    continue
                nc.gpsimd.collective_compute(
                    "AllGather", ALU.bypass, ins=[mshard.opt()],
                    outs=[mfull.opt()], replica_groups=rg_msgs)
                # w^T wrap for the NEXT step's received matmuls; modH is
                # kept alive in its own buffer, so this runs inside the
                # AllGather window instead of on the pre-AG critical path.
                # wTblk[32g+k, g, b, c] = sig(w[b, 4c+g, k])
                for g in range(4):
                    for ch in range(2):
                        ps = ps1p.tile([128, 512], F32, tag="mm1")
                        for bi in range(2):
                            b = 2 * ch + bi
                            for m in range(2):
                                nc.tensor.matmul(
                                    ps[0:K, 256 * bi:256 * (bi + 1)],
                                    t_dw2wT[:, m, :],
                                    modHr[:, m, b, :, g],
                                    start=(m == 0), stop=(m == 1),
                                    skip_group_check=True)
                        nc.scalar.activation(
                            out=wTblk[32 * g:32 * (g + 1), g,
                                      2 * ch:2 * (ch + 1), :],
                            in_=ps[0:K, :].rearrange("p (b c) -> p b c", b=2),
                            func=AF.Sigmoid, bias=t_db2wT[:])

    nc.finalize()
    return nc


def _prep_inputs(inputs):
    """Build the per-core input maps from the full problem inputs."""
    cc = np.asarray(inputs["cc_signals"], dtype=np.float32)
    h0 = np.asarray(inputs["h0"], dtype=np.float32)
    msgs0 = np.asarray(inputs["msgs0"], dtype=np.float32)
    w_conn0 = np.asarray(inputs["w_conn0"], dtype=np.float32)
    hebb = np.asarray(inputs["hebbian"], dtype=np.float32)
    ident = np.asarray(inputs["identity"], dtype=np.float32)
    conn = np.asarray(inputs["conn_indices"]).astype(np.int64)

    def f32(x):
        return np.ascontiguousarray(x, dtype=np.float32)

    def bf16(x):
        return np.ascontiguousarray(
            np.asarray(x, dtype=np.float32).astype(ml_dtypes.bfloat16))

    dw1 = np.asarray(inputs["dw1"], dtype=np.float32)   # [256, 256]
    dw2 = np.asarray(inputs["dw2"], dtype=np.float32)   # [256, 65]
    db2 = np.asarray(inputs["db2"], dtype=np.float32)   # [65]
    sw1 = np.asarray(inputs["sw1"], dtype=np.float32)   # [224, 256]
    sw2 = np.asarray(inputs["sw2"], dtype=np.float32)   # [256, 64]
    mw1 = np.asarray(inputs["mw1"], dtype=np.float32)   # [96, 256]
    mw2 = np.asarray(inputs["mw2"], dtype=np.float32)   # [256, 64]

    # dw1 input order: [hebb(0:32), h(32:96), ide(96:128), rcv, inj]
    # C rows: [h, ide, hebb]; B rows: [rcv, inj]
    shared = {
        "dw1C": f32(np.concatenate([dw1[32:96], dw1[96:128], dw1[0:32]])),
        "dw1B": f32(dw1[128:256]),
        "db1": f32(np.asarray(inputs["db1"]).reshape(2, 128).T),
        "dw2wT": f32(dw2[:, 0:K].reshape(2, 128, K).transpose(1, 0, 2)),
        "db2wT": f32(db2[0:K].reshape(K, 1)),
        "dw2om": f32(np.repeat(dw2[:, K:K + 1], D, axis=1)
                      .reshape(2, 128, D).transpose(1, 0, 2)),
        "db2om": f32(np.full((D, 1), -db2[K])),
        "dw2de": f32(dw2[:, K + 1:].reshape(2, 128, D_ID).transpose(1, 0, 2)),
        "db2de": f32(4.0 * db2[K + 1:].reshape(D_ID, 1)),
        "sw1B": f32(sw1[0:128]),
        "sw1C": f32(sw1[128:224]),
        "sb1": f32(np.asarray(inputs["sb1"]).reshape(2, 128).T),
        "sw2": f32(sw2.reshape(2, 128, D).transpose(1, 0, 2)),
        "sb2": f32(np.asarray(inputs["sb2"]).reshape(D, 1)),
        "mw1": f32(mw1),
        "mb1": f32(np.asarray(inputs["mb1"]).reshape(2, 128).T),
        "mw2": np.ascontiguousarray(mw2.reshape(2, 128, D).transpose(1, 0, 2), dtype=np.float16),
        "mb2": np.ascontiguousarray(np.tile(np.asarray(inputs["mb2"]).reshape(1, D), (1, BS)), dtype=np.float16),
        "ones1": np.ascontiguousarray(np.ones((1, 128)), dtype=np.float16),
    }

    def sigmoid(x):
        return 1.0 / (1.0 + np.exp(-x))

    def f16(x):
        return np.ascontiguousarray(x, dtype=np.float16)

    seg = cc.reshape(BS, T, N // 512, D)  # [b, t, slice, d]
    mfull0_full = f16(msgs0.transpose(1, 0, 2).reshape(N, EL))
    in_maps = []
    for c in range(NCORES):
        bsl = slice(0, BS)
        sh = slice(c * NT, (c + 1) * NT)
        m = dict(shared)
        m["h0T"] = f32(h0[bsl, sh].transpose(2, 0, 1).reshape(D, R))
        m["hebbT"] = f32(hebb[bsl, sh].transpose(2, 0, 1).reshape(D_ID, R))
        m["identT"] = f32(ident[sh].T)

        injT = np.empty((T, D, NB, NT), dtype=np.float32)
        for q in range(2):
            injT[:, :, :, 512 * q:512 * (q + 1)] = \
                seg[bsl, :, 2 * c + q].transpose(1, 2, 0)[:, :, :, None]
        m["injT"] = f32(injT.reshape(T, D, R))

        # full msgs0, batch-interleaved [n, b, d] (read by step-0 gathers)
        m["mfull0"] = mfull0_full

        # block-diag wrapped sigmoid(w0): blk[32g+k, g, b, c] = s(w0[b,4c+g,k])
        w0 = sigmoid(w_conn0[bsl, sh])          # [NB, NT, K]
        wr = w0.reshape(NB, NT // 4, 4, K)      # [b, c, g, k]
        blk = np.zeros((128, 4, NB, NT // 4), dtype=np.float32)
        for g in range(4):
            blk[32 * g:32 * (g + 1), g] = wr[:, :, g, :].transpose(2, 0, 1)
        m["w0blk"] = f16(blk)

        # gather indices: instr g covers targets 32g..32g+32;
        # lin[i] for i = 1024*g + 128*cp + 32*gp + k  -> conn[4*(8g+cp)+gp, k]
        tgt = conn[sh]                          # [NT, K] global ids
        lin = tgt.reshape(NT // 4, 4, K).reshape(NG, 8, 4, K).reshape(-1)
        wrapped = lin.reshape(2048, 16).T.astype(np.int16)   # [16, 2048]
        m["idx"] = np.ascontiguousarray(np.tile(wrapped, (8, 1)))
        in_maps.append(m)
    return in_maps


def kernel(**inputs) -> np.ndarray:
    key = "prog"
    if key not in _PROGRAM_CACHE:
        _PROGRAM_CACHE[key] = _build_program()
    nc = _PROGRAM_CACHE[key]

    in_maps = _prep_inputs(inputs)
    res = run_bass_kernel_spmd(nc, in_maps, list(range(NCORES)))
    full = np.empty((BS, T, N, D), dtype=np.float32)
    for c in range(NCORES):
        o = np.asarray(res.results[c]["out"]).astype(np.float32)  # [T, D, R]
        o = o.reshape(T, D, NB, NT).transpose(2, 0, 3, 1)
        full[:, :, c * NT:(c + 1) * NT, :] = o
    return full.reshape(BS, T, N // 64, 64 * D)

